# revision 3
# baseline (speedup 1.0000x reference)
"""MoE routing kernel for Trainium2 (8 NeuronCores).

Reference computation (B=16384, IN=64, HID=128, OUT=1, E=64, NMAP=1000):
    e = c[num]                                  # [B] expert id per sample
    h = relu(x @ W1[e] + b1[e])                 # [B, HID]
    y = sigmoid(h @ W2[e] + b2[e])              # [B, OUT]

Strategy: sort-by-expert dispatch on the host (the routing is pure
integer bookkeeping), dense per-expert matmuls on device. Each core gets
the same static slot structure (SPMD: one graph for all 8 cores); slot
widths are equalized across cores by snake-dealing the per-expert chunks
in descending size order, padding each slot to the max width over cores.

Per-core device graph, per slot j (width Wj <= 512):
    mm1:  psum1[HID=128, Wj] = W1_j[64,128].T @ xT[64, Wj]
    act1: h[128, Wj] = relu(psum1 + b1_j)        (ScalarE, per-partition bias)
    mm2:  psum2[1, Wj] = w2_j[128,1].T @ h[128, Wj]
    act2: y[1, Wj] = sigmoid(psum2 + b2_j)
All inputs stay resident in SBUF (~1 MB/core), one DMA per tensor.
"""

import sys

if "/opt/trn_rl_repo" not in sys.path:
    sys.path.insert(0, "/opt/trn_rl_repo")

import numpy as np

import concourse.bass as bass
import concourse.mybir as mybir
from concourse import tile
from concourse.bass_utils import run_bass_kernel_spmd

N_CORES = 8
IN = 64
HID = 128
E = 64
MAX_W = 512  # fp32 moving-operand / PSUM-bank limit


# ---------------------------------------------------------------------------
# This container's walrus build rejects more than one sync wait per
# instruction ("Too many sync wait commands"). Post-pass over the lowered
# BIR: move the extra waits onto single-wait NOPs inserted just before the
# instruction on the same engine (program order makes this equivalent).
# ---------------------------------------------------------------------------
_MAX_WAITS = 1


def _split_multi_waits(nc):
    ctr = 0
    for f in nc.m.functions:
        for blk in f.blocks:
            new_list = []
            for ins in blk.instructions:
                si = ins.sync_info
                if si is not None and si.on_wait and len(si.on_wait) > _MAX_WAITS:
                    waits = list(si.on_wait)
                    head, tail = waits[:-_MAX_WAITS], waits[-_MAX_WAITS:]
                    for i in range(0, len(head), _MAX_WAITS):
                        ctr += 1
                        new_list.append(
                            mybir.InstNoOp(
                                name=f"waitsplit-{ctr}",
                                engine=ins.engine,
                                bass_nofuse=True,
                                sync_info=mybir.SyncInfo(
                                    on_wait=head[i : i + _MAX_WAITS], on_update=[]
                                ),
                            )
                        )
                    si.on_wait = tail
                    ins.sync_info = si
                new_list.append(ins)
            blk.instructions = new_list


# ---------------------------------------------------------------------------
# Host-side routing: build the per-core slot structure.
# ---------------------------------------------------------------------------
def _plan(e: np.ndarray):
    """Return (slot_widths, per_core_slots) where per_core_slots[i] is a list
    of (expert_id, sample_indices) aligned with slot_widths."""
    order = np.argsort(e, kind="stable")
    counts = np.bincount(e, minlength=E)
    starts = np.concatenate([[0], np.cumsum(counts)])

    chunks = []  # (width, expert, indices)
    for ex in range(E):
        idx = order[starts[ex] : starts[ex + 1]]
        for pos in range(0, len(idx), MAX_W):
            sub = idx[pos : pos + MAX_W]
            chunks.append((len(sub), ex, sub))
    chunks.sort(key=lambda t: -t[0])

    per_core = [[] for _ in range(N_CORES)]
    for r in range(0, len(chunks), N_CORES):
        row = chunks[r : r + N_CORES]
        cores = range(N_CORES) if (r // N_CORES) % 2 == 0 else range(N_CORES - 1, -1, -1)
        for ch, core in zip(row, cores):
            per_core[core].append(ch)

    n_slots = max(len(s) for s in per_core)
    empty = np.zeros((0,), dtype=np.int64)
    for s in per_core:
        while len(s) < n_slots:
            s.append((0, 0, empty))
        s.sort(key=lambda t: -t[0])

    widths = [max(per_core[i][j][0] for i in range(N_CORES)) for j in range(n_slots)]
    widths = [max(w, 1) for w in widths]
    slots = [[(s[j][1], s[j][2]) for j in range(n_slots)] for s in per_core]
    return widths, slots


# ---------------------------------------------------------------------------
# Device graph builder (shared by all cores).
# ---------------------------------------------------------------------------
def _build(widths):
    S = len(widths)
    NT = int(sum(widths))
    f32 = mybir.dt.float32

    nc = bass.Bass("TRN2", target_bir_lowering=False, debug=False)
    xt_e = nc.declare_dram_parameter("xt", [IN, NT], f32, isOutput=False)
    w1_e = nc.declare_dram_parameter("w1", [IN, S * HID], f32, isOutput=False)
    w2_e = nc.declare_dram_parameter("w2", [HID, S], f32, isOutput=False)
    b1_e = nc.declare_dram_parameter("b1", [HID, S], f32, isOutput=False)
    b2_e = nc.declare_dram_parameter("b2", [1, S], f32, isOutput=False)
    y_e = nc.declare_dram_parameter("y", [1, NT], f32, isOutput=True)

    relu = mybir.ActivationFunctionType.Relu
    sigmoid = mybir.ActivationFunctionType.Sigmoid

    with tile.TileContext(nc) as tc:
        with (
            tc.tile_pool(name="sb", bufs=1) as sb,
            tc.tile_pool(name="hp", bufs=3) as hp,
            tc.tile_pool(name="ps1", bufs=2, space="PSUM") as ps1,
            tc.tile_pool(name="ps2", bufs=2, space="PSUM") as ps2,
        ):
            xt = sb.tile([IN, NT], f32)
            w1 = sb.tile([IN, S * HID], f32)
            w2 = sb.tile([HID, S], f32)
            b1 = sb.tile([HID, S], f32)
            b2 = sb.tile([1, S], f32)
            y = sb.tile([1, NT], f32)
            nc.sync.dma_start(xt[:], xt_e[:])
            nc.sync.dma_start(w1[:], w1_e[:])
            nc.sync.dma_start(w2[:], w2_e[:])
            nc.sync.dma_start(b1[:], b1_e[:])
            nc.sync.dma_start(b2[:], b2_e[:])

            off = 0
            for j, wj in enumerate(widths):
                p1 = ps1.tile([HID, wj], f32, tag="p1")
                nc.tensor.matmul(
                    p1[:],
                    w1[:, j * HID : (j + 1) * HID],
                    xt[:, off : off + wj],
                    start=True,
                    stop=True,
                )
                h = hp.tile([HID, wj], f32, tag="h")
                nc.scalar.activation(h[:], p1[:], relu, bias=b1[:, j : j + 1])
                p2 = ps2.tile([1, wj], f32, tag="p2")
                nc.tensor.matmul(
                    p2[:], w2[:, j : j + 1], h[:], start=True, stop=True
                )
                nc.scalar.activation(
                    y[:, off : off + wj], p2[:], sigmoid, bias=b2[:, j : j + 1]
                )
                off += wj

            nc.sync.dma_start(y_e[:], y[:])

    _split_multi_waits(nc)
    return nc


# ---------------------------------------------------------------------------
# Entry point.
# ---------------------------------------------------------------------------
def _run(inputs, trace=False):
    x = np.asarray(inputs["x"], dtype=np.float32)
    num = np.asarray(inputs["num"])
    c = np.asarray(inputs["c"])
    W1 = np.asarray(inputs["W1"], dtype=np.float32)
    b1 = np.asarray(inputs["b1"], dtype=np.float32)
    W2 = np.asarray(inputs["W2"], dtype=np.float32)
    b2 = np.asarray(inputs["b2"], dtype=np.float32)

    B = x.shape[0]
    e = c[num].astype(np.int64)
    widths, slots = _plan(e)
    S = len(widths)
    NT = int(sum(widths))
    offs = np.concatenate([[0], np.cumsum(widths)]).astype(np.int64)

    in_maps = []
    for core in range(N_CORES):
        xt_c = np.zeros((IN, NT), dtype=np.float32)
        w1_c = np.empty((IN, S * HID), dtype=np.float32)
        w2_c = np.empty((HID, S), dtype=np.float32)
        b1_c = np.empty((HID, S), dtype=np.float32)
        b2_c = np.empty((1, S), dtype=np.float32)
        for j in range(S):
            ex, idx = slots[core][j]
            if len(idx):
                xt_c[:, offs[j] : offs[j] + len(idx)] = x[idx].T
            w1_c[:, j * HID : (j + 1) * HID] = W1[ex]
            w2_c[:, j] = W2[ex, :, 0]
            b1_c[:, j] = b1[ex]
            b2_c[0, j] = b2[ex, 0]
        in_maps.append(
            {"xt": xt_c, "w1": w1_c, "w2": w2_c, "b1": b1_c, "b2": b2_c}
        )

    nc = _build(widths)
    res = run_bass_kernel_spmd(nc, in_maps, list(range(N_CORES)), trace=trace)

    out = np.empty((B, 1), dtype=np.float32)
    for core in range(N_CORES):
        y_c = res.results[core]["y"]
        for j in range(S):
            ex, idx = slots[core][j]
            if len(idx):
                out[idx, 0] = y_c[0, offs[j] : offs[j] + len(idx)]
    return out, res


def kernel(**inputs) -> np.ndarray:
    out, _ = _run(inputs, trace=False)
    return out


# revision 4
# speedup vs baseline: 1.5997x; 1.5997x over previous
"""MoE routing kernel for Trainium2 (8 NeuronCores).

Reference computation (B=16384, IN=64, HID=128, OUT=1, E=64, NMAP=1000):
    e = c[num]                                  # [B] expert id per sample
    h = relu(x @ W1[e] + b1[e])                 # [B, HID]
    y = sigmoid(h @ W2[e] + b2[e])              # [B, OUT]

Strategy: sort-by-expert dispatch on the host (the routing is pure
integer bookkeeping), dense per-expert matmuls on device. Each core gets
the same static slot structure (SPMD: one graph for all 8 cores); slot
widths are equalized across cores by snake-dealing the per-expert chunks
in descending size order, padding each slot to the max width over cores.

Device data layout (per core): slots are paired onto the 128 SBUF
partitions — pair p puts slot 2p's x^T on partitions 0:64 and slot
2p+1's on 64:128. This gives full-width DMA and lets the two K=64
matmuls of a pair run concurrently in disjoint PE row groups. All
tensor data is bf16 (rel-err budget 2e-2); accumulation stays f32.

Per slot j (width Wj <= 512):
    mm1:   psum1[HID=128, Wj] = W1_j[64,128].T @ xT[64, Wj]     (PE)
    relu:  h[128, Wj] = bf16(max(psum1 + b1_j, 0))              (DVE)
    mm2:   psum2[1, Wj] = w2_j[128,1].T @ h[128, Wj]            (PE)
    sig:   y[1, Wj] = sigmoid(psum2 + b2_j)                     (ACT)
"""

import sys

if "/opt/trn_rl_repo" not in sys.path:
    sys.path.insert(0, "/opt/trn_rl_repo")

import numpy as np

import concourse.bass as bass
import concourse.mybir as mybir
from concourse import tile
from concourse.bass_utils import run_bass_kernel_spmd

N_CORES = 8
IN = 64
HID = 128
E = 64
MAX_W = 512  # moving-operand / PSUM-bank limit

BF16 = mybir.dt.bfloat16
F32 = mybir.dt.float32
NP_BF16 = mybir.dt.np(BF16)


# ---------------------------------------------------------------------------
# This container's walrus build rejects more than one sync wait per
# instruction ("Too many sync wait commands"). Post-pass over the lowered
# BIR: move the extra waits onto single-wait NOPs inserted just before the
# instruction on the same engine (program order makes this equivalent).
# ---------------------------------------------------------------------------
_MAX_WAITS = 1


def _split_multi_waits(nc):
    ctr = 0
    for f in nc.m.functions:
        for blk in f.blocks:
            new_list = []
            for ins in blk.instructions:
                si = ins.sync_info
                if si is not None and si.on_wait and len(si.on_wait) > _MAX_WAITS:
                    waits = list(si.on_wait)
                    head, tail = waits[:-_MAX_WAITS], waits[-_MAX_WAITS:]
                    for i in range(0, len(head), _MAX_WAITS):
                        ctr += 1
                        new_list.append(
                            mybir.InstNoOp(
                                name=f"waitsplit-{ctr}",
                                engine=ins.engine,
                                bass_nofuse=True,
                                sync_info=mybir.SyncInfo(
                                    on_wait=head[i : i + _MAX_WAITS], on_update=[]
                                ),
                            )
                        )
                    si.on_wait = tail
                    ins.sync_info = si
                new_list.append(ins)
            blk.instructions = new_list


# ---------------------------------------------------------------------------
# Host-side routing: build the per-core slot structure.
# ---------------------------------------------------------------------------
def _plan(e: np.ndarray):
    """Return (slot_widths, per_core_slots) where per_core_slots[i] is a list
    of (expert_id, sample_indices) aligned with slot_widths (desc order)."""
    order = np.argsort(e, kind="stable")
    counts = np.bincount(e, minlength=E)
    starts = np.concatenate([[0], np.cumsum(counts)])

    chunks = []  # (width, expert, indices)
    for ex in range(E):
        idx = order[starts[ex] : starts[ex + 1]]
        for pos in range(0, len(idx), MAX_W):
            sub = idx[pos : pos + MAX_W]
            chunks.append((len(sub), ex, sub))
    chunks.sort(key=lambda t: -t[0])

    per_core = [[] for _ in range(N_CORES)]
    for r in range(0, len(chunks), N_CORES):
        row = chunks[r : r + N_CORES]
        cores = range(N_CORES) if (r // N_CORES) % 2 == 0 else range(N_CORES - 1, -1, -1)
        for ch, core in zip(row, cores):
            per_core[core].append(ch)

    n_slots = max(len(s) for s in per_core)
    empty = np.zeros((0,), dtype=np.int64)
    for s in per_core:
        while len(s) < n_slots:
            s.append((0, 0, empty))
        s.sort(key=lambda t: -t[0])

    widths = [max(per_core[i][j][0] for i in range(N_CORES)) for j in range(n_slots)]
    widths = [max(w, 1) for w in widths]
    slots = [[(s[j][1], s[j][2]) for j in range(n_slots)] for s in per_core]
    return widths, slots


def _layout(widths):
    """Column layout. Slots are paired; pair p spans widths[2p] columns of
    the packed xT region (slot 2p on partitions 0:64, slot 2p+1 on 64:128).
    Returns (pair_offs, y_offs, NTP, NT, P, S)."""
    S = len(widths)
    P = (S + 1) // 2
    pws = [widths[2 * p] for p in range(P)]
    pair_offs = np.concatenate([[0], np.cumsum(pws)]).astype(np.int64)
    y_offs = np.concatenate([[0], np.cumsum(widths)]).astype(np.int64)
    return pair_offs, y_offs, int(pair_offs[-1]), int(y_offs[-1]), P, S


# ---------------------------------------------------------------------------
# Device graph builder (shared by all cores).
# ---------------------------------------------------------------------------
def _build(widths):
    pair_offs, y_offs, NTP, NT, P, S = _layout(widths)
    # data tensor columns: [0, NTP) packed xT | [NTP, NTP+P*HID) packed W1
    # | [NTP+P*HID, +S) w2 columns
    W1_OFF = NTP
    W2_OFF = NTP + P * HID
    DCOLS = W2_OFF + S

    nc = bass.Bass("TRN2", target_bir_lowering=False, debug=False)
    data_e = nc.declare_dram_parameter("data", [128, DCOLS], BF16, isOutput=False)
    bias_e = nc.declare_dram_parameter("bias", [128, 2 * S], F32, isOutput=False)
    y_e = nc.declare_dram_parameter("y", [1, NT], F32, isOutput=True)

    sigmoid = mybir.ActivationFunctionType.Sigmoid
    add = mybir.AluOpType.add
    amax = mybir.AluOpType.max

    with tile.TileContext(nc) as tc:
        with (
            tc.tile_pool(name="sb", bufs=1) as sb,
            tc.tile_pool(name="hp", bufs=4) as hp,
            tc.tile_pool(name="ps1", bufs=4, space="PSUM") as ps1,
            tc.tile_pool(name="ps2", bufs=2, space="PSUM") as ps2,
        ):
            data = sb.tile([128, DCOLS], BF16)
            bias = sb.tile([128, 2 * S], F32)
            y = sb.tile([1, NT], F32)
            nc.sync.dma_start(data[:], data_e[:])
            nc.sync.dma_start(bias[:], bias_e[:])

            def slot_aps(j):
                p, hi = divmod(j, 2)
                r0 = 64 * hi
                wj = widths[j]
                xt = data[r0 : r0 + 64, pair_offs[p] : pair_offs[p] + wj]
                w1 = data[r0 : r0 + 64, W1_OFF + p * HID : W1_OFF + (p + 1) * HID]
                return xt, w1

            def mm1(j):
                wj = widths[j]
                xt, w1 = slot_aps(j)
                p1 = ps1.tile([HID, wj], F32, tag="p1")
                nc.tensor.matmul(p1[:], w1, xt, start=True, stop=True)
                return p1

            def relu(j, p1):
                wj = widths[j]
                h = hp.tile([HID, wj], BF16, tag="h")
                nc.vector.tensor_scalar(
                    h[:], p1[:], bias[:, j : j + 1], 0.0, add, amax
                )
                return h

            def mm2(j, h):
                wj = widths[j]
                p2 = ps2.tile([1, wj], F32, tag="p2")
                nc.tensor.matmul(
                    p2[:], data[:, W2_OFF + j : W2_OFF + j + 1], h[:],
                    start=True, stop=True,
                )
                return p2

            def sig(j, p2):
                wj = widths[j]
                nc.scalar.activation(
                    y[:, y_offs[j] : y_offs[j] + wj], p2[:], sigmoid,
                    bias=bias[0:1, S + j : S + j + 1],
                )

            # software-pipelined emission: mm1 of pair p+1 runs on PE while
            # DVE does relu of pair p; mm2 of pair p follows.
            stage = []  # (j, p1)
            for p in range(P + 1):
                if p < P:
                    js = [2 * p] + ([2 * p + 1] if 2 * p + 1 < S else [])
                    nxt = [(j, mm1(j)) for j in js]
                else:
                    nxt = []
                for j, p1 in stage:
                    h = relu(j, p1)
                    p2 = mm2(j, h)
                    sig(j, p2)
                stage = nxt

            nc.sync.dma_start(y_e[:], y[:])

    _split_multi_waits(nc)
    return nc


# ---------------------------------------------------------------------------
# Entry point.
# ---------------------------------------------------------------------------
def _run(inputs, trace=False):
    x = np.asarray(inputs["x"], dtype=np.float32)
    num = np.asarray(inputs["num"])
    c = np.asarray(inputs["c"])
    W1 = np.asarray(inputs["W1"], dtype=np.float32)
    b1 = np.asarray(inputs["b1"], dtype=np.float32)
    W2 = np.asarray(inputs["W2"], dtype=np.float32)
    b2 = np.asarray(inputs["b2"], dtype=np.float32)

    B = x.shape[0]
    e = c[num].astype(np.int64)
    widths, slots = _plan(e)
    pair_offs, y_offs, NTP, NT, P, S = _layout(widths)
    W1_OFF = NTP
    W2_OFF = NTP + P * HID
    DCOLS = W2_OFF + S

    x_bf = x.astype(NP_BF16)
    W1_bf = W1.astype(NP_BF16)
    W2_bf = W2.astype(NP_BF16)

    in_maps = []
    for core in range(N_CORES):
        data_c = np.zeros((128, DCOLS), dtype=NP_BF16)
        bias_c = np.zeros((128, 2 * S), dtype=np.float32)
        for j in range(S):
            ex, idx = slots[core][j]
            p, hi = divmod(j, 2)
            r0 = 64 * hi
            if len(idx):
                data_c[r0 : r0 + 64, pair_offs[p] : pair_offs[p] + len(idx)] = (
                    x_bf[idx].T
                )
            data_c[r0 : r0 + 64, W1_OFF + p * HID : W1_OFF + (p + 1) * HID] = (
                W1_bf[ex]
            )
            data_c[:, W2_OFF + j] = W2_bf[ex, :, 0]
            bias_c[:, j] = b1[ex]
            bias_c[0, S + j] = b2[ex, 0]
        in_maps.append({"data": data_c, "bias": bias_c})

    nc = _build(widths)
    res = run_bass_kernel_spmd(nc, in_maps, list(range(N_CORES)), trace=trace)

    out = np.empty((B, 1), dtype=np.float32)
    for core in range(N_CORES):
        y_c = res.results[core]["y"]
        for j in range(S):
            ex, idx = slots[core][j]
            if len(idx):
                out[idx, 0] = y_c[0, y_offs[j] : y_offs[j] + len(idx)]
    return out, res


def kernel(**inputs) -> np.ndarray:
    out, _ = _run(inputs, trace=False)
    return out


# revision 10
# speedup vs baseline: 1.6934x; 1.0585x over previous
"""MoE routing kernel for Trainium2 (8 NeuronCores).

Reference computation (B=16384, IN=64, HID=128, OUT=1, E=64, NMAP=1000):
    e = c[num]                                  # [B] expert id per sample
    h = relu(x @ W1[e] + b1[e])                 # [B, HID]
    y = sigmoid(h @ W2[e] + b2[e])              # [B, OUT]

Strategy: sort-by-expert dispatch on the host (the routing is pure
integer bookkeeping), dense per-expert matmuls on device. Each core gets
the same static slot structure (SPMD: one graph for all 8 cores); slot
widths are equalized across cores by snake-dealing the per-expert chunks
in descending size order, padding each slot to the max width over cores.

Device data layout (per core): slots are paired onto the 128 SBUF
partitions — pair p puts slot 2p's x^T on partitions 0:64 and slot
2p+1's on 64:128. This gives full-width DMA and lets the two K=64
matmuls of a pair run concurrently in disjoint PE row groups. All
tensor data is bf16 (rel-err budget 2e-2); accumulation stays f32.

Per slot j (width Wj <= 512):
    mm1:   psum1[HID=128, Wj] = W1_j[64,128].T @ xT[64, Wj]     (PE)
    relu:  h[128, Wj] = bf16(max(psum1 + b1_j, 0))              (DVE)
    mm2:   psum2[1, Wj] = w2_j[128,1].T @ h[128, Wj]            (PE)
    sig:   y[1, Wj] = sigmoid(psum2 + b2_j)                     (ACT)
"""

import sys

if "/opt/trn_rl_repo" not in sys.path:
    sys.path.insert(0, "/opt/trn_rl_repo")

import numpy as np

import concourse.bass as bass
import concourse.mybir as mybir
from concourse import tile
from concourse.bass_utils import run_bass_kernel_spmd

N_CORES = 8
IN = 64
HID = 128
E = 64
MAX_W = 512  # moving-operand / PSUM-bank limit

BF16 = mybir.dt.bfloat16
F32 = mybir.dt.float32
NP_BF16 = mybir.dt.np(BF16)


# ---------------------------------------------------------------------------
# This container's walrus build rejects more than one sync wait per
# instruction ("Too many sync wait commands"). Post-pass over the lowered
# BIR: move the extra waits onto single-wait NOPs inserted just before the
# instruction on the same engine (program order makes this equivalent).
# ---------------------------------------------------------------------------
_MAX_WAITS = 1


def _split_multi_waits(nc):
    ctr = 0
    for f in nc.m.functions:
        for blk in f.blocks:
            new_list = []
            for ins in blk.instructions:
                si = ins.sync_info
                if si is not None and si.on_wait and len(si.on_wait) > _MAX_WAITS:
                    waits = list(si.on_wait)
                    head, tail = waits[:-_MAX_WAITS], waits[-_MAX_WAITS:]
                    for i in range(0, len(head), _MAX_WAITS):
                        ctr += 1
                        new_list.append(
                            mybir.InstNoOp(
                                name=f"waitsplit-{ctr}",
                                engine=ins.engine,
                                bass_nofuse=True,
                                sync_info=mybir.SyncInfo(
                                    on_wait=head[i : i + _MAX_WAITS], on_update=[]
                                ),
                            )
                        )
                    si.on_wait = tail
                    ins.sync_info = si
                new_list.append(ins)
            blk.instructions = new_list


# ---------------------------------------------------------------------------
# Host-side routing: build the per-core slot structure.
# ---------------------------------------------------------------------------
def _plan(e: np.ndarray):
    """Return (slot_widths, per_core_slots) where per_core_slots[i] is a list
    of (expert_id, sample_indices) aligned with slot_widths (desc order)."""
    order = np.argsort(e, kind="stable")
    counts = np.bincount(e, minlength=E)
    starts = np.concatenate([[0], np.cumsum(counts)])

    chunks = []  # (width, expert, indices)
    for ex in range(E):
        idx = order[starts[ex] : starts[ex + 1]]
        for pos in range(0, len(idx), MAX_W):
            sub = idx[pos : pos + MAX_W]
            chunks.append((len(sub), ex, sub))
    chunks.sort(key=lambda t: -t[0])

    per_core = [[] for _ in range(N_CORES)]
    for r in range(0, len(chunks), N_CORES):
        row = chunks[r : r + N_CORES]
        cores = range(N_CORES) if (r // N_CORES) % 2 == 0 else range(N_CORES - 1, -1, -1)
        for ch, core in zip(row, cores):
            per_core[core].append(ch)

    n_slots = max(len(s) for s in per_core)
    empty = np.zeros((0,), dtype=np.int64)
    for s in per_core:
        while len(s) < n_slots:
            s.append((0, 0, empty))
        s.sort(key=lambda t: -t[0])

    widths = [max(per_core[i][j][0] for i in range(N_CORES)) for j in range(n_slots)]
    widths = [max(w, 1) for w in widths]
    slots = [[(s[j][1], s[j][2]) for j in range(n_slots)] for s in per_core]
    return widths, slots


def _layout(widths):
    """Column layout. Slots are paired; pair p spans widths[2p] columns of
    the packed xT region (slot 2p on partitions 0:64, slot 2p+1 on 64:128).
    Returns (pair_offs, y_offs, NTP, NT, P, S)."""
    S = len(widths)
    P = (S + 1) // 2
    pws = [widths[2 * p] for p in range(P)]
    pair_offs = np.concatenate([[0], np.cumsum(pws)]).astype(np.int64)
    y_offs = np.concatenate([[0], np.cumsum(widths)]).astype(np.int64)
    return pair_offs, y_offs, int(pair_offs[-1]), int(y_offs[-1]), P, S


# ---------------------------------------------------------------------------
# Device graph builder (shared by all cores).
# ---------------------------------------------------------------------------
def _build(widths):
    pair_offs, y_offs, NTP, NT, P, S = _layout(widths)
    # data tensor columns: [0, P*HID) packed W1 | [P*HID, +S) w2 columns
    # | [W1W2, W1W2+NTP) packed xT
    W2_OFF = P * HID
    XT_OFF = W2_OFF + S
    DCOLS = XT_OFF + NTP
    # first input DMA covers weights + the first SPLIT pairs' xT columns
    SPLIT = min(2, P)
    CUT = int(XT_OFF + pair_offs[SPLIT])
    # output DMA split after the first YSPLIT slots
    YSPLIT = min(2 * SPLIT, S)
    YCUT = int(y_offs[YSPLIT])

    nc = bass.Bass("TRN2", target_bir_lowering=False, debug=False)
    data_e = nc.declare_dram_parameter("data", [128, DCOLS], BF16, isOutput=False)
    bias_e = nc.declare_dram_parameter("bias", [128, 2 * S], F32, isOutput=False)
    y_e = nc.declare_dram_parameter("y", [1, NT], F32, isOutput=True)

    sigmoid = mybir.ActivationFunctionType.Sigmoid
    add = mybir.AluOpType.add
    amax = mybir.AluOpType.max

    with tile.TileContext(nc) as tc:
        with (
            tc.tile_pool(name="sb", bufs=1) as sb,
            tc.tile_pool(name="hp", bufs=4) as hp,
            tc.tile_pool(name="ps1", bufs=4, space="PSUM") as ps1,
            tc.tile_pool(name="ps2", bufs=2, space="PSUM") as ps2,
            tc.tile_pool(name="dummy", bufs=1) as dummy_pool,
            tc.tile_pool(name="psd", bufs=1, space="PSUM") as psd,
        ):
            # ACT sigmoid table preload + PE HAM warmup during the input
            # DMA window: both run on garbage SBUF with no data deps.
            import os
            WARMUP = os.environ.get("K_WARMUP", "1") == "1"
            if WARMUP:
                warm = dummy_pool.tile([128, 512], BF16)
                warm_in = dummy_pool.tile([1, 16], F32)
                warm_ps = psd.tile([128, 448], F32)
                warm_y = dummy_pool.tile([1, 16], F32)
                nc.gpsimd.memset(warm[:], 0.0)
                nc.gpsimd.memset(warm_in[:], 0.0)
                nc.scalar.activation(warm_y[:], warm_in[:], sigmoid)
                for _ in range(8):
                    nc.tensor.matmul(
                        warm_ps[:], warm[:, :128], warm[:, :448],
                        start=True, stop=True,
                    )

            data1 = sb.tile([128, CUT], BF16)
            data2 = sb.tile([128, DCOLS - CUT], BF16)
            bias = sb.tile([128, 2 * S], F32)
            y1 = sb.tile([1, YCUT], F32)
            y2 = sb.tile([1, NT - YCUT], F32)
            nc.sync.dma_start(bias[:], bias_e[:])
            nc.sync.dma_start(data1[:], data_e[:, :CUT])
            nc.sync.dma_start(data2[:], data_e[:, CUT:])

            def dcols(c0, c1, r0=0, r1=128):
                if c1 <= CUT:
                    return data1[r0:r1, c0:c1]
                assert c0 >= CUT
                return data2[r0:r1, c0 - CUT : c1 - CUT]

            def yslice(c0, c1):
                if c1 <= YCUT:
                    return y1[:, c0:c1]
                assert c0 >= YCUT
                return y2[:, c0 - YCUT : c1 - YCUT]

            def slot_aps(j):
                p, hi = divmod(j, 2)
                r0 = 64 * hi
                wj = widths[j]
                c0 = XT_OFF + int(pair_offs[p])
                xt = dcols(c0, c0 + wj, r0, r0 + 64)
                w1 = dcols(p * HID, (p + 1) * HID, r0, r0 + 64)
                return xt, w1

            def mm1(j):
                wj = widths[j]
                xt, w1 = slot_aps(j)
                p1 = ps1.tile([HID, wj], F32, tag="p1")
                nc.tensor.matmul(p1[:], w1, xt, start=True, stop=True)
                return p1

            def relu(j, p1):
                wj = widths[j]
                h = hp.tile([HID, wj], BF16, tag="h")
                nc.vector.tensor_scalar(
                    h[:], p1[:], bias[:, j : j + 1], 0.0, add, amax
                )
                return h

            def mm2(j, h):
                wj = widths[j]
                p2 = ps2.tile([1, wj], F32, tag="p2")
                nc.tensor.matmul(
                    p2[:], dcols(W2_OFF + j, W2_OFF + j + 1), h[:],
                    start=True, stop=True,
                )
                return p2

            def sig(j, p2):
                wj = widths[j]
                nc.scalar.activation(
                    yslice(int(y_offs[j]), int(y_offs[j]) + wj), p2[:], sigmoid,
                    bias=bias[0:1, S + j : S + j + 1],
                )

            # software-pipelined emission: mm1 of pair p+1 runs on PE while
            # DVE does relu of pair p; mm2 of pair p follows.
            stage = []  # (j, p1)
            for p in range(P + 1):
                if p < P:
                    js = [2 * p] + ([2 * p + 1] if 2 * p + 1 < S else [])
                    nxt = [(j, mm1(j)) for j in js]
                else:
                    nxt = []
                for j, p1 in stage:
                    h = relu(j, p1)
                    p2 = mm2(j, h)
                    sig(j, p2)
                if p == P - 1:
                    # first chunk of the output leaves while the tail computes
                    nc.sync.dma_start(y_e[:, :YCUT], y1[:])
                stage = nxt

            nc.sync.dma_start(y_e[:, YCUT:], y2[:])

    _split_multi_waits(nc)
    return nc


# ---------------------------------------------------------------------------
# Entry point.
# ---------------------------------------------------------------------------
def _run(inputs, trace=False):
    x = np.asarray(inputs["x"], dtype=np.float32)
    num = np.asarray(inputs["num"])
    c = np.asarray(inputs["c"])
    W1 = np.asarray(inputs["W1"], dtype=np.float32)
    b1 = np.asarray(inputs["b1"], dtype=np.float32)
    W2 = np.asarray(inputs["W2"], dtype=np.float32)
    b2 = np.asarray(inputs["b2"], dtype=np.float32)

    B = x.shape[0]
    e = c[num].astype(np.int64)
    widths, slots = _plan(e)
    pair_offs, y_offs, NTP, NT, P, S = _layout(widths)
    W2_OFF = P * HID
    XT_OFF = W2_OFF + S
    DCOLS = XT_OFF + NTP

    x_bf = x.astype(NP_BF16)
    W1_bf = W1.astype(NP_BF16)
    W2_bf = W2.astype(NP_BF16)

    in_maps = []
    for core in range(N_CORES):
        data_c = np.zeros((128, DCOLS), dtype=NP_BF16)
        bias_c = np.zeros((128, 2 * S), dtype=np.float32)
        for j in range(S):
            ex, idx = slots[core][j]
            p, hi = divmod(j, 2)
            r0 = 64 * hi
            if len(idx):
                data_c[
                    r0 : r0 + 64,
                    XT_OFF + pair_offs[p] : XT_OFF + pair_offs[p] + len(idx),
                ] = x_bf[idx].T
            data_c[r0 : r0 + 64, p * HID : (p + 1) * HID] = W1_bf[ex]
            data_c[:, W2_OFF + j] = W2_bf[ex, :, 0]
            bias_c[:, j] = b1[ex]
            bias_c[0, S + j] = b2[ex, 0]
        in_maps.append({"data": data_c, "bias": bias_c})

    nc = _build(widths)
    res = run_bass_kernel_spmd(nc, in_maps, list(range(N_CORES)), trace=trace)

    out = np.empty((B, 1), dtype=np.float32)
    for core in range(N_CORES):
        y_c = res.results[core]["y"]
        for j in range(S):
            ex, idx = slots[core][j]
            if len(idx):
                out[idx, 0] = y_c[0, y_offs[j] : y_offs[j] + len(idx)]
    return out, res


def kernel(**inputs) -> np.ndarray:
    out, _ = _run(inputs, trace=False)
    return out


# revision 20
# speedup vs baseline: 1.7327x; 1.0232x over previous
"""MoE routing kernel for Trainium2 (8 NeuronCores).

Reference computation (B=16384, IN=64, HID=128, OUT=1, E=64, NMAP=1000):
    e = c[num]                                  # [B] expert id per sample
    h = relu(x @ W1[e] + b1[e])                 # [B, HID]
    y = sigmoid(h @ W2[e] + b2[e])              # [B, OUT]

Strategy: sort-by-expert dispatch on the host (the routing is pure
integer bookkeeping), dense per-expert matmuls on device. Each core gets
the same static slot structure (SPMD: one graph for all 8 cores); slot
widths are equalized across cores by snake-dealing the per-expert chunks
in descending size order, padding each slot to the max width over cores.

Device data layout (per core): slots are paired onto the 128 SBUF
partitions — pair p puts slot 2p's x^T on partitions 0:64 and slot
2p+1's on 64:128. This gives full-width DMA and lets the two K=64
matmuls of a pair run concurrently in disjoint PE row groups. All
tensor data is bf16 (rel-err budget 2e-2); accumulation stays f32.

Per slot j (width Wj <= 512):
    mm1:   psum1[HID=128, Wj] = W1_j[64,128].T @ xT[64, Wj]     (PE)
    relu:  h[128, Wj] = bf16(max(psum1 + b1_j, 0))              (DVE)
    mm2:   psum2[1, Wj] = w2_j[128,1].T @ h[128, Wj]            (PE)
    sig:   y[1, Wj] = sigmoid(psum2 + b2_j)                     (ACT)
"""

import sys

if "/opt/trn_rl_repo" not in sys.path:
    sys.path.insert(0, "/opt/trn_rl_repo")

import numpy as np

import concourse.bass as bass
import concourse.mybir as mybir
from concourse import tile
from concourse.bass_utils import run_bass_kernel_spmd

N_CORES = 8
IN = 64
HID = 128
E = 64
MAX_W = 512  # moving-operand / PSUM-bank limit

BF16 = mybir.dt.bfloat16
F32 = mybir.dt.float32
NP_BF16 = mybir.dt.np(BF16)


# ---------------------------------------------------------------------------
# This container's walrus build rejects more than one sync wait per
# instruction ("Too many sync wait commands"). Post-pass over the lowered
# BIR: move the extra waits onto single-wait NOPs inserted just before the
# instruction on the same engine (program order makes this equivalent).
# ---------------------------------------------------------------------------
_MAX_WAITS = 1


def _split_multi_waits(nc):
    ctr = 0
    for f in nc.m.functions:
        for blk in f.blocks:
            new_list = []
            for ins in blk.instructions:
                si = ins.sync_info
                if si is not None and si.on_wait and len(si.on_wait) > _MAX_WAITS:
                    waits = list(si.on_wait)
                    head, tail = waits[:-_MAX_WAITS], waits[-_MAX_WAITS:]
                    for i in range(0, len(head), _MAX_WAITS):
                        ctr += 1
                        new_list.append(
                            mybir.InstNoOp(
                                name=f"waitsplit-{ctr}",
                                engine=ins.engine,
                                bass_nofuse=True,
                                sync_info=mybir.SyncInfo(
                                    on_wait=head[i : i + _MAX_WAITS], on_update=[]
                                ),
                            )
                        )
                    si.on_wait = tail
                    ins.sync_info = si
                new_list.append(ins)
            blk.instructions = new_list


# ---------------------------------------------------------------------------
# Host-side routing: build the per-core slot structure.
# ---------------------------------------------------------------------------
def _plan(e: np.ndarray):
    """Return (slot_widths, per_core_slots) where per_core_slots[i] is a list
    of (expert_id, sample_indices) aligned with slot_widths (desc order)."""
    order = np.argsort(e, kind="stable")
    counts = np.bincount(e, minlength=E)
    starts = np.concatenate([[0], np.cumsum(counts)])

    chunks = []  # (width, expert, indices)
    for ex in range(E):
        idx = order[starts[ex] : starts[ex + 1]]
        for pos in range(0, len(idx), MAX_W):
            sub = idx[pos : pos + MAX_W]
            chunks.append((len(sub), ex, sub))
    chunks.sort(key=lambda t: -t[0])

    per_core = [[] for _ in range(N_CORES)]
    for r in range(0, len(chunks), N_CORES):
        row = chunks[r : r + N_CORES]
        cores = range(N_CORES) if (r // N_CORES) % 2 == 0 else range(N_CORES - 1, -1, -1)
        for ch, core in zip(row, cores):
            per_core[core].append(ch)

    n_slots = max(len(s) for s in per_core)
    empty = np.zeros((0,), dtype=np.int64)
    for s in per_core:
        while len(s) < n_slots:
            s.append((0, 0, empty))
        s.sort(key=lambda t: -t[0])

    widths = [max(per_core[i][j][0] for i in range(N_CORES)) for j in range(n_slots)]
    widths = [max(w, 1) for w in widths]
    slots = [[(s[j][1], s[j][2]) for j in range(n_slots)] for s in per_core]
    return widths, slots


def _layout(widths, bin_cap=MAX_W):
    """Column layout. Slots are paired; pair p spans widths[2p] columns of
    the packed xT region (slot 2p on partitions 0:64, slot 2p+1 on 64:128).
    Slots are also first-fit packed into "bins" of <=bin_cap y columns;
    each bin is one PSUM bank for the mm2 outputs and one sigmoid
    instruction (bin_cap<=0: one slot per bin, used when b2 varies).
    Returns (pair_offs, NTP, NT, P, S, bins, bin_off, slot_bin, slot_y_off)."""
    S = len(widths)
    P = (S + 1) // 2
    pws = [widths[2 * p] for p in range(P)]
    pair_offs = np.concatenate([[0], np.cumsum(pws)]).astype(np.int64)
    NT = int(np.sum(widths))

    bins = []  # list of [slot indices]
    bin_w = []
    slot_bin = [0] * S
    for j in range(S):
        for b in range(len(bins)):
            if bin_cap > 0 and bin_w[b] + widths[j] <= bin_cap:
                bins[b].append(j)
                bin_w[b] += widths[j]
                slot_bin[j] = b
                break
        else:
            slot_bin[j] = len(bins)
            bins.append([j])
            bin_w.append(widths[j])

    slot_y_off = [0] * S
    off = 0
    bin_off = []
    for b, bslots in enumerate(bins):
        bin_off.append(off)
        for j in bslots:
            slot_y_off[j] = off
            off += widths[j]
    assert off == NT
    return pair_offs, int(pair_offs[-1]), NT, P, S, bins, bin_off, slot_bin, slot_y_off


# ---------------------------------------------------------------------------
# Device graph builder (shared by all cores).
# ---------------------------------------------------------------------------
def _build(widths, b2_uniform):
    pair_offs, NTP, NT, P, S, bins, bin_off, slot_bin, slot_y_off = _layout(
        widths, MAX_W if b2_uniform else 0
    )
    # data tensor columns: [0, P*HID) packed W1 | [P*HID, +S) w2 columns
    # | [W1W2, W1W2+NTP) packed xT
    W2_OFF = P * HID
    XT_OFF = W2_OFF + S
    DCOLS = XT_OFF + NTP
    # first input DMA covers weights + the first SPLIT pairs' xT columns
    SPLIT = min(2, P)
    CUT = int(XT_OFF + pair_offs[SPLIT])
    # output DMA split: y1 covers the first YBINS bins
    YBINS = max(1, len(bins) // 2) if len(bins) > 1 else 0
    YCUT = int(bin_off[YBINS]) if YBINS else 0

    nc = bass.Bass("TRN2", target_bir_lowering=False, debug=False)
    data_e = nc.declare_dram_parameter("data", [128, DCOLS], BF16, isOutput=False)
    bias_e = nc.declare_dram_parameter("bias", [128, 2 * S], F32, isOutput=False)
    y_e = nc.declare_dram_parameter("y", [1, NT], F32, isOutput=True)

    sigmoid = mybir.ActivationFunctionType.Sigmoid
    add = mybir.AluOpType.add
    amax = mybir.AluOpType.max

    NBINS = len(bins)
    with tile.TileContext(nc) as tc:
        with (
            tc.tile_pool(name="sb", bufs=1) as sb,
            tc.tile_pool(name="hp", bufs=4) as hp,
            tc.tile_pool(name="ps1", bufs=3, space="PSUM") as ps1,
            tc.tile_pool(name="ps2", bufs=1, space="PSUM") as ps2,
            tc.tile_pool(name="dummy", bufs=1) as dummy_pool,
        ):
            # ACT sigmoid table preload + PE HAM warmup during the input
            # DMA window: both run on garbage SBUF with no data deps. The
            # warmup matmuls rotate through the same psum bufs the real
            # mm1s use (PE executes in order, so no hazard).
            import os
            WARMUP = os.environ.get("K_WARMUP", "1") == "1"
            if WARMUP:
                warm = dummy_pool.tile([128, 512], BF16)
                warm_in = dummy_pool.tile([1, 16], F32)
                warm_y = dummy_pool.tile([1, 16], F32)
                nc.gpsimd.memset(warm[:], 0.0)
                nc.gpsimd.memset(warm_in[:], 0.0)
                nc.scalar.activation(warm_y[:], warm_in[:], sigmoid)
                for _ in range(8):
                    warm_ps = ps1.tile([HID, 448], F32, tag="p1")
                    nc.tensor.matmul(
                        warm_ps[:], warm[:, :128], warm[:, :448],
                        start=True, stop=True,
                    )

            data1 = sb.tile([128, CUT], BF16)
            data2 = sb.tile([128, DCOLS - CUT], BF16)
            bias = sb.tile([128, 2 * S], F32)
            y1 = sb.tile([1, YCUT], F32)
            y2 = sb.tile([1, NT - YCUT], F32)
            nc.sync.dma_start(data1[:], data_e[:, :CUT])
            nc.sync.dma_start(data2[:], data_e[:, CUT:])
            nc.sync.dma_start(bias[:], bias_e[:])

            def dcols(c0, c1, r0=0, r1=128):
                if c1 <= CUT:
                    return data1[r0:r1, c0:c1]
                assert c0 >= CUT
                return data2[r0:r1, c0 - CUT : c1 - CUT]

            def yslice(c0, c1):
                if c1 <= YCUT:
                    return y1[:, c0:c1]
                assert c0 >= YCUT
                return y2[:, c0 - YCUT : c1 - YCUT]

            def slot_aps(j):
                p, hi = divmod(j, 2)
                r0 = 64 * hi
                wj = widths[j]
                c0 = XT_OFF + int(pair_offs[p])
                xt = dcols(c0, c0 + wj, r0, r0 + 64)
                w1 = dcols(p * HID, (p + 1) * HID, r0, r0 + 64)
                return xt, w1

            def mm1(j):
                wj = widths[j]
                xt, w1 = slot_aps(j)
                p1 = ps1.tile([HID, wj], F32, tag="p1")
                nc.tensor.matmul(p1[:], w1, xt, start=True, stop=True)
                return p1

            def relu(j, p1):
                wj = widths[j]
                h = hp.tile([HID, wj], BF16, tag="h")
                nc.vector.tensor_scalar(
                    h[:], p1[:], bias[:, j : j + 1], 0.0, add, amax
                )
                return h

            # one PSUM bank per bin; mm2 of each slot writes its column
            # range, one sigmoid per bin reads the whole bank.
            bin_ps = []
            for b in range(NBINS):
                bw = int(sum(widths[j] for j in bins[b]))
                bin_tile = ps2.tile([1, bw], F32, tag=f"bin{b}")
                bin_ps.append(bin_tile)
            bin_left = [len(bs) for bs in bins]

            def mm2(j, h):
                wj = widths[j]
                b = slot_bin[j]
                c0 = int(slot_y_off[j] - bin_off[b])
                nc.tensor.matmul(
                    bin_ps[b][:, c0 : c0 + wj],
                    dcols(W2_OFF + j, W2_OFF + j + 1), h[:],
                    start=True, stop=True,
                )

            def sig_bin(b):
                c0 = int(bin_off[b])
                wb = bin_ps[b].shape[-1]
                if b2_uniform:
                    bias_ap = bias[0:1, S : S + 1]
                else:
                    bias_ap = bias[0:1, S + bins[b][0] : S + bins[b][0] + 1]
                nc.scalar.activation(
                    yslice(c0, c0 + wb), bin_ps[b][:], sigmoid, bias=bias_ap
                )

            def finish_slot(j):
                b = slot_bin[j]
                bin_left[b] -= 1
                if bin_left[b] == 0:
                    sig_bin(b)
                    if YCUT and all(
                        bin_left[bb] == 0 for bb in range(YBINS)
                    ) and b < YBINS:
                        nc.sync.dma_start(y_e[:, :YCUT], y1[:])

            # software-pipelined emission: mm1 of pair p+1 runs on PE while
            # DVE does relu of pair p; mm2 of pair p follows.
            stage = []  # (j, p1)
            for p in range(P + 1):
                if p < P:
                    js = [2 * p] + ([2 * p + 1] if 2 * p + 1 < S else [])
                    nxt = [(j, mm1(j)) for j in js]
                else:
                    nxt = []
                for j, p1 in stage:
                    h = relu(j, p1)
                    mm2(j, h)
                    finish_slot(j)
                stage = nxt

            if YCUT and any(bin_left[bb] > 0 for bb in range(YBINS)):
                nc.sync.dma_start(y_e[:, :YCUT], y1[:])
            if YCUT:
                nc.sync.dma_start(y_e[:, YCUT:], y2[:])
            else:
                nc.sync.dma_start(y_e[:], y2[:])

    _split_multi_waits(nc)
    return nc


# ---------------------------------------------------------------------------
# Entry point.
# ---------------------------------------------------------------------------
def _run(inputs, trace=False):
    x = np.asarray(inputs["x"], dtype=np.float32)
    num = np.asarray(inputs["num"])
    c = np.asarray(inputs["c"])
    W1 = np.asarray(inputs["W1"], dtype=np.float32)
    b1 = np.asarray(inputs["b1"], dtype=np.float32)
    W2 = np.asarray(inputs["W2"], dtype=np.float32)
    b2 = np.asarray(inputs["b2"], dtype=np.float32)

    B = x.shape[0]
    e = c[num].astype(np.int64)
    b2_uniform = bool(np.all(b2 == b2.flat[0]))
    widths, slots = _plan(e)
    pair_offs, NTP, NT, P, S, bins, bin_off, slot_bin, slot_y_off = _layout(
        widths, MAX_W if b2_uniform else 0
    )
    W2_OFF = P * HID
    XT_OFF = W2_OFF + S
    DCOLS = XT_OFF + NTP

    x_bf = x.astype(NP_BF16)
    W1_bf = W1.astype(NP_BF16)
    W2_bf = W2.astype(NP_BF16)

    in_maps = []
    for core in range(N_CORES):
        data_c = np.zeros((128, DCOLS), dtype=NP_BF16)
        bias_c = np.zeros((128, 2 * S), dtype=np.float32)
        for j in range(S):
            ex, idx = slots[core][j]
            p, hi = divmod(j, 2)
            r0 = 64 * hi
            if len(idx):
                data_c[
                    r0 : r0 + 64,
                    XT_OFF + pair_offs[p] : XT_OFF + pair_offs[p] + len(idx),
                ] = x_bf[idx].T
            data_c[r0 : r0 + 64, p * HID : (p + 1) * HID] = W1_bf[ex]
            data_c[:, W2_OFF + j] = W2_bf[ex, :, 0]
            bias_c[:, j] = b1[ex]
            bias_c[0, S + j] = b2[ex, 0]
        bias_c[0, S] = b2.flat[0] if b2_uniform else bias_c[0, S]
        in_maps.append({"data": data_c, "bias": bias_c})

    nc = _build(widths, b2_uniform)
    res = run_bass_kernel_spmd(nc, in_maps, list(range(N_CORES)), trace=trace)

    out = np.empty((B, 1), dtype=np.float32)
    for core in range(N_CORES):
        y_c = res.results[core]["y"]
        for j in range(S):
            ex, idx = slots[core][j]
            if len(idx):
                out[idx, 0] = y_c[0, slot_y_off[j] : slot_y_off[j] + len(idx)]
    return out, res


def kernel(**inputs) -> np.ndarray:
    out, _ = _run(inputs, trace=False)
    return out


# revision 22
# speedup vs baseline: 1.7436x; 1.0063x over previous
"""MoE routing kernel for Trainium2 (8 NeuronCores).

Reference computation (B=16384, IN=64, HID=128, OUT=1, E=64, NMAP=1000):
    e = c[num]                                  # [B] expert id per sample
    h = relu(x @ W1[e] + b1[e])                 # [B, HID]
    y = sigmoid(h @ W2[e] + b2[e])              # [B, OUT]

Strategy: sort-by-expert dispatch on the host (the routing is pure
integer bookkeeping), dense per-expert matmuls on device. Each core gets
the same static slot structure (SPMD: one graph for all 8 cores); slot
widths are equalized across cores by snake-dealing the per-expert chunks
in descending size order, padding each slot to the max width over cores.

Device data layout (per core): slots are paired onto the 128 SBUF
partitions — pair p puts slot 2p's x^T on partitions 0:64 and slot
2p+1's on 64:128. This gives full-width DMA and lets the two K=64
matmuls of a pair run concurrently in disjoint PE row groups. All
tensor data is bf16 (rel-err budget 2e-2); accumulation stays f32.

Per slot j (width Wj <= 512):
    mm1:   psum1[HID=128, Wj] = W1_j[64,128].T @ xT[64, Wj]     (PE)
    relu:  h[128, Wj] = bf16(max(psum1 + b1_j, 0))              (DVE)
    mm2:   psum2[1, Wj] = w2_j[128,1].T @ h[128, Wj]            (PE)
    sig:   y[1, Wj] = sigmoid(psum2 + b2_j)                     (ACT)
"""

import sys

if "/opt/trn_rl_repo" not in sys.path:
    sys.path.insert(0, "/opt/trn_rl_repo")

import numpy as np

import concourse.bass as bass
import concourse.mybir as mybir
from concourse import tile
from concourse.bass_utils import run_bass_kernel_spmd

N_CORES = 8
IN = 64
HID = 128
E = 64
MAX_W = 512  # moving-operand / PSUM-bank limit

BF16 = mybir.dt.bfloat16
F32 = mybir.dt.float32
NP_BF16 = mybir.dt.np(BF16)


# ---------------------------------------------------------------------------
# This container's walrus build rejects more than one sync wait per
# instruction ("Too many sync wait commands"). Post-pass over the lowered
# BIR: move the extra waits onto single-wait NOPs inserted just before the
# instruction on the same engine (program order makes this equivalent).
# ---------------------------------------------------------------------------
_MAX_WAITS = 1


def _split_multi_waits(nc):
    ctr = 0
    for f in nc.m.functions:
        for blk in f.blocks:
            new_list = []
            for ins in blk.instructions:
                si = ins.sync_info
                if si is not None and si.on_wait and len(si.on_wait) > _MAX_WAITS:
                    waits = list(si.on_wait)
                    head, tail = waits[:-_MAX_WAITS], waits[-_MAX_WAITS:]
                    for i in range(0, len(head), _MAX_WAITS):
                        ctr += 1
                        new_list.append(
                            mybir.InstNoOp(
                                name=f"waitsplit-{ctr}",
                                engine=ins.engine,
                                bass_nofuse=True,
                                sync_info=mybir.SyncInfo(
                                    on_wait=head[i : i + _MAX_WAITS], on_update=[]
                                ),
                            )
                        )
                    si.on_wait = tail
                    ins.sync_info = si
                new_list.append(ins)
            blk.instructions = new_list


# ---------------------------------------------------------------------------
# Host-side routing: build the per-core slot structure.
# ---------------------------------------------------------------------------
def _plan(e: np.ndarray):
    """Return (slot_widths, per_core_slots) where per_core_slots[i] is a list
    of (expert_id, sample_indices) aligned with slot_widths (desc order)."""
    order = np.argsort(e, kind="stable")
    counts = np.bincount(e, minlength=E)
    starts = np.concatenate([[0], np.cumsum(counts)])

    chunks = []  # (width, expert, indices)
    for ex in range(E):
        idx = order[starts[ex] : starts[ex + 1]]
        for pos in range(0, len(idx), MAX_W):
            sub = idx[pos : pos + MAX_W]
            chunks.append((len(sub), ex, sub))
    chunks.sort(key=lambda t: -t[0])

    per_core = [[] for _ in range(N_CORES)]
    for r in range(0, len(chunks), N_CORES):
        row = chunks[r : r + N_CORES]
        cores = range(N_CORES) if (r // N_CORES) % 2 == 0 else range(N_CORES - 1, -1, -1)
        for ch, core in zip(row, cores):
            per_core[core].append(ch)

    n_slots = max(len(s) for s in per_core)
    empty = np.zeros((0,), dtype=np.int64)
    for s in per_core:
        while len(s) < n_slots:
            s.append((0, 0, empty))
        s.sort(key=lambda t: -t[0])

    widths = [max(per_core[i][j][0] for i in range(N_CORES)) for j in range(n_slots)]
    widths = [max(w, 1) for w in widths]
    slots = [[(s[j][1], s[j][2]) for j in range(n_slots)] for s in per_core]
    return widths, slots


def _layout(widths, bin_cap=MAX_W):
    """Column layout. Slots are paired; pair p spans widths[2p] columns of
    the packed xT region (slot 2p on partitions 0:64, slot 2p+1 on 64:128).
    Slots are also first-fit packed into "bins" of <=bin_cap y columns;
    each bin is one PSUM bank for the mm2 outputs and one sigmoid
    instruction (bin_cap<=0: one slot per bin, used when b2 varies).
    Returns (pair_offs, NTP, NT, P, S, bins, bin_off, slot_bin, slot_y_off)."""
    S = len(widths)
    P = (S + 1) // 2
    pws = [widths[2 * p] for p in range(P)]
    pair_offs = np.concatenate([[0], np.cumsum(pws)]).astype(np.int64)
    NT = int(np.sum(widths))

    bins = []  # list of [slot indices]
    bin_w = []
    slot_bin = [0] * S
    for j in range(S):
        for b in range(len(bins)):
            if bin_cap > 0 and bin_w[b] + widths[j] <= bin_cap:
                bins[b].append(j)
                bin_w[b] += widths[j]
                slot_bin[j] = b
                break
        else:
            slot_bin[j] = len(bins)
            bins.append([j])
            bin_w.append(widths[j])

    slot_y_off = [0] * S
    off = 0
    bin_off = []
    for b, bslots in enumerate(bins):
        bin_off.append(off)
        for j in bslots:
            slot_y_off[j] = off
            off += widths[j]
    assert off == NT
    return pair_offs, int(pair_offs[-1]), NT, P, S, bins, bin_off, slot_bin, slot_y_off


# ---------------------------------------------------------------------------
# Device graph builder (shared by all cores).
# ---------------------------------------------------------------------------
def _build(widths, b2_uniform):
    pair_offs, NTP, NT, P, S, bins, bin_off, slot_bin, slot_y_off = _layout(
        widths, MAX_W if b2_uniform else 0
    )
    # data tensor columns: [0, P*HID) packed W1 | [P*HID, +S) w2 columns
    # | [W1W2, W1W2+NTP) packed xT
    W2_OFF = P * HID
    XT_OFF = W2_OFF + S
    DCOLS = XT_OFF + NTP
    # first input DMA covers weights + the first SPLIT pairs' xT columns
    SPLIT = min(2, P)
    CUT = int(XT_OFF + pair_offs[SPLIT])
    # output DMA split: y1 covers the first YBINS bins
    YBINS = max(1, len(bins) // 2) if len(bins) > 1 else 0
    YCUT = int(bin_off[YBINS]) if YBINS else 0

    nc = bass.Bass("TRN2", target_bir_lowering=False, debug=False)
    data_e = nc.declare_dram_parameter("data", [128, DCOLS], BF16, isOutput=False)
    bias_e = nc.declare_dram_parameter("bias", [128, 2 * S], F32, isOutput=False)
    y_e = nc.declare_dram_parameter("y", [1, NT], F32, isOutput=True)

    sigmoid = mybir.ActivationFunctionType.Sigmoid
    add = mybir.AluOpType.add
    amax = mybir.AluOpType.max

    NBINS = len(bins)
    with tile.TileContext(nc) as tc:
        with (
            tc.tile_pool(name="sb", bufs=1) as sb,
            tc.tile_pool(name="hp", bufs=4) as hp,
            tc.tile_pool(name="ps1", bufs=3, space="PSUM") as ps1,
            tc.tile_pool(name="ps2", bufs=1, space="PSUM") as ps2,
            tc.tile_pool(name="dummy", bufs=1) as dummy_pool,
        ):
            # ACT sigmoid table preload + PE HAM warmup during the input
            # DMA window: both run on garbage SBUF with no data deps. The
            # warmup matmuls rotate through the same psum bufs the real
            # mm1s use (PE executes in order, so no hazard).
            import os
            WARMUP = os.environ.get("K_WARMUP", "1") == "1"
            if WARMUP:
                warm = dummy_pool.tile([128, 512], BF16)
                warm_in = dummy_pool.tile([1, 16], F32)
                warm_y = dummy_pool.tile([1, 16], F32)
                nc.gpsimd.memset(warm[:], 0.0)
                nc.gpsimd.memset(warm_in[:], 0.0)
                nc.scalar.activation(warm_y[:], warm_in[:], sigmoid)
                for _ in range(5):
                    warm_ps = ps1.tile([HID, 448], F32, tag="p1")
                    nc.tensor.matmul(
                        warm_ps[:], warm[:, :128], warm[:, :448],
                        start=True, stop=True,
                    )

            data1 = sb.tile([128, CUT], BF16)
            data2 = sb.tile([128, DCOLS - CUT], BF16)
            bias = sb.tile([128, 2 * S], F32)
            y1 = sb.tile([1, YCUT], F32)
            y2 = sb.tile([1, NT - YCUT], F32)
            HALF = CUT // 2
            nc.sync.dma_start(data1[:, :HALF], data_e[:, :HALF])
            nc.scalar.dma_start(data1[:, HALF:], data_e[:, HALF:CUT])
            nc.sync.dma_start(data2[:], data_e[:, CUT:])
            nc.scalar.dma_start(bias[:], bias_e[:])

            def dcols(c0, c1, r0=0, r1=128):
                if c1 <= CUT:
                    return data1[r0:r1, c0:c1]
                assert c0 >= CUT
                return data2[r0:r1, c0 - CUT : c1 - CUT]

            def yslice(c0, c1):
                if c1 <= YCUT:
                    return y1[:, c0:c1]
                assert c0 >= YCUT
                return y2[:, c0 - YCUT : c1 - YCUT]

            def slot_aps(j):
                p, hi = divmod(j, 2)
                r0 = 64 * hi
                wj = widths[j]
                c0 = XT_OFF + int(pair_offs[p])
                xt = dcols(c0, c0 + wj, r0, r0 + 64)
                w1 = dcols(p * HID, (p + 1) * HID, r0, r0 + 64)
                return xt, w1

            def mm1(j):
                wj = widths[j]
                xt, w1 = slot_aps(j)
                p1 = ps1.tile([HID, wj], F32, tag="p1")
                nc.tensor.matmul(p1[:], w1, xt, start=True, stop=True)
                return p1

            def relu(j, p1):
                wj = widths[j]
                h = hp.tile([HID, wj], BF16, tag="h")
                nc.vector.tensor_scalar(
                    h[:], p1[:], bias[:, j : j + 1], 0.0, add, amax
                )
                return h

            # one PSUM bank per bin; mm2 of each slot writes its column
            # range, one sigmoid per bin reads the whole bank.
            bin_ps = []
            for b in range(NBINS):
                bw = int(sum(widths[j] for j in bins[b]))
                bin_tile = ps2.tile([1, bw], F32, tag=f"bin{b}")
                bin_ps.append(bin_tile)
            bin_left = [len(bs) for bs in bins]

            def mm2(j, h):
                wj = widths[j]
                b = slot_bin[j]
                c0 = int(slot_y_off[j] - bin_off[b])
                nc.tensor.matmul(
                    bin_ps[b][:, c0 : c0 + wj],
                    dcols(W2_OFF + j, W2_OFF + j + 1), h[:],
                    start=True, stop=True,
                )

            def sig_bin(b):
                c0 = int(bin_off[b])
                wb = bin_ps[b].shape[-1]
                if b2_uniform:
                    bias_ap = bias[0:1, S : S + 1]
                else:
                    bias_ap = bias[0:1, S + bins[b][0] : S + bins[b][0] + 1]
                nc.scalar.activation(
                    yslice(c0, c0 + wb), bin_ps[b][:], sigmoid, bias=bias_ap
                )

            def finish_slot(j):
                b = slot_bin[j]
                bin_left[b] -= 1
                if bin_left[b] == 0:
                    sig_bin(b)
                    if YCUT and all(
                        bin_left[bb] == 0 for bb in range(YBINS)
                    ) and b < YBINS:
                        nc.sync.dma_start(y_e[:, :YCUT], y1[:])

            # software-pipelined emission: mm1 of pair p+1 runs on PE while
            # DVE does relu of pair p; mm2 of pair p follows.
            stage = []  # (j, p1)
            for p in range(P + 1):
                if p < P:
                    js = [2 * p] + ([2 * p + 1] if 2 * p + 1 < S else [])
                    nxt = [(j, mm1(j)) for j in js]
                else:
                    nxt = []
                for j, p1 in stage:
                    h = relu(j, p1)
                    mm2(j, h)
                    finish_slot(j)
                stage = nxt

            if YCUT and any(bin_left[bb] > 0 for bb in range(YBINS)):
                nc.sync.dma_start(y_e[:, :YCUT], y1[:])
            if YCUT:
                nc.sync.dma_start(y_e[:, YCUT:], y2[:])
            else:
                nc.sync.dma_start(y_e[:], y2[:])

    _split_multi_waits(nc)
    return nc


# ---------------------------------------------------------------------------
# Entry point.
# ---------------------------------------------------------------------------
def _run(inputs, trace=False):
    x = np.asarray(inputs["x"], dtype=np.float32)
    num = np.asarray(inputs["num"])
    c = np.asarray(inputs["c"])
    W1 = np.asarray(inputs["W1"], dtype=np.float32)
    b1 = np.asarray(inputs["b1"], dtype=np.float32)
    W2 = np.asarray(inputs["W2"], dtype=np.float32)
    b2 = np.asarray(inputs["b2"], dtype=np.float32)

    B = x.shape[0]
    e = c[num].astype(np.int64)
    b2_uniform = bool(np.all(b2 == b2.flat[0]))
    widths, slots = _plan(e)
    pair_offs, NTP, NT, P, S, bins, bin_off, slot_bin, slot_y_off = _layout(
        widths, MAX_W if b2_uniform else 0
    )
    W2_OFF = P * HID
    XT_OFF = W2_OFF + S
    DCOLS = XT_OFF + NTP

    x_bf = x.astype(NP_BF16)
    W1_bf = W1.astype(NP_BF16)
    W2_bf = W2.astype(NP_BF16)

    in_maps = []
    for core in range(N_CORES):
        data_c = np.zeros((128, DCOLS), dtype=NP_BF16)
        bias_c = np.zeros((128, 2 * S), dtype=np.float32)
        for j in range(S):
            ex, idx = slots[core][j]
            p, hi = divmod(j, 2)
            r0 = 64 * hi
            if len(idx):
                data_c[
                    r0 : r0 + 64,
                    XT_OFF + pair_offs[p] : XT_OFF + pair_offs[p] + len(idx),
                ] = x_bf[idx].T
            data_c[r0 : r0 + 64, p * HID : (p + 1) * HID] = W1_bf[ex]
            data_c[:, W2_OFF + j] = W2_bf[ex, :, 0]
            bias_c[:, j] = b1[ex]
            bias_c[0, S + j] = b2[ex, 0]
        bias_c[0, S] = b2.flat[0] if b2_uniform else bias_c[0, S]
        in_maps.append({"data": data_c, "bias": bias_c})

    nc = _build(widths, b2_uniform)
    res = run_bass_kernel_spmd(nc, in_maps, list(range(N_CORES)), trace=trace)

    out = np.empty((B, 1), dtype=np.float32)
    for core in range(N_CORES):
        y_c = res.results[core]["y"]
        for j in range(S):
            ex, idx = slots[core][j]
            if len(idx):
                out[idx, 0] = y_c[0, slot_y_off[j] : slot_y_off[j] + len(idx)]
    return out, res


def kernel(**inputs) -> np.ndarray:
    out, _ = _run(inputs, trace=False)
    return out


# revision 23
# speedup vs baseline: 1.7887x; 1.0259x over previous
"""MoE routing kernel for Trainium2 (8 NeuronCores).

Reference computation (B=16384, IN=64, HID=128, OUT=1, E=64, NMAP=1000):
    e = c[num]                                  # [B] expert id per sample
    h = relu(x @ W1[e] + b1[e])                 # [B, HID]
    y = sigmoid(h @ W2[e] + b2[e])              # [B, OUT]

Strategy: sort-by-expert dispatch on the host (the routing is pure
integer bookkeeping), dense per-expert matmuls on device. Each core gets
the same static slot structure (SPMD: one graph for all 8 cores); slot
widths are equalized across cores by snake-dealing the per-expert chunks
in descending size order, padding each slot to the max width over cores.

Device data layout (per core): slots are paired onto the 128 SBUF
partitions — pair p puts slot 2p's x^T on partitions 0:64 and slot
2p+1's on 64:128. This gives full-width DMA and lets the two K=64
matmuls of a pair run concurrently in disjoint PE row groups. All
tensor data is bf16 (rel-err budget 2e-2); accumulation stays f32.

Per slot j (width Wj <= 512):
    mm1:   psum1[HID=128, Wj] = W1_j[64,128].T @ xT[64, Wj]     (PE)
    relu:  h[128, Wj] = bf16(max(psum1 + b1_j, 0))              (DVE)
    mm2:   psum2[1, Wj] = w2_j[128,1].T @ h[128, Wj]            (PE)
    sig:   y[1, Wj] = sigmoid(psum2 + b2_j)                     (ACT)
"""

import sys

if "/opt/trn_rl_repo" not in sys.path:
    sys.path.insert(0, "/opt/trn_rl_repo")

import numpy as np

import concourse.bass as bass
import concourse.mybir as mybir
from concourse import tile
from concourse.bass_utils import run_bass_kernel_spmd

N_CORES = 8
IN = 64
HID = 128
E = 64
MAX_W = 512  # moving-operand / PSUM-bank limit

BF16 = mybir.dt.bfloat16
F32 = mybir.dt.float32
NP_BF16 = mybir.dt.np(BF16)


# ---------------------------------------------------------------------------
# This container's walrus build rejects more than one sync wait per
# instruction ("Too many sync wait commands"). Post-pass over the lowered
# BIR: move the extra waits onto single-wait NOPs inserted just before the
# instruction on the same engine (program order makes this equivalent).
# ---------------------------------------------------------------------------
_MAX_WAITS = 1


def _slim_drain_and_barrier(self, tick_clock, wait_clock):
    """Replacement for TileContext._drain_and_barrier: the NEFF here runs
    exactly once per load (run_bass_via_pjrt → single execute), so skip
    the semaphore re-zeroing and the second barrier, and use the
    sequencer-only barrier (no per-engine InstDrain flushes)."""
    drain_inst = self.nc.sync.drain()
    wait_clock.add_sem_waits(
        drain_inst.ins, tile.ScopedClock({None: tick_clock.global_clock})
    )
    self.nc.all_engine_barrier(sem_only=True)
    popped = self.nc._tile_sem_poison_stack.pop()
    assert popped is self._sem_poison


tile.TileContext._drain_and_barrier = _slim_drain_and_barrier


def _split_multi_waits(nc):
    ctr = 0
    for f in nc.m.functions:
        for blk in f.blocks:
            new_list = []
            for ins in blk.instructions:
                si = ins.sync_info
                if si is not None and si.on_wait and len(si.on_wait) > _MAX_WAITS:
                    waits = list(si.on_wait)
                    head, tail = waits[:-_MAX_WAITS], waits[-_MAX_WAITS:]
                    for i in range(0, len(head), _MAX_WAITS):
                        ctr += 1
                        new_list.append(
                            mybir.InstNoOp(
                                name=f"waitsplit-{ctr}",
                                engine=ins.engine,
                                bass_nofuse=True,
                                sync_info=mybir.SyncInfo(
                                    on_wait=head[i : i + _MAX_WAITS], on_update=[]
                                ),
                            )
                        )
                    si.on_wait = tail
                    ins.sync_info = si
                new_list.append(ins)
            blk.instructions = new_list


# ---------------------------------------------------------------------------
# Host-side routing: build the per-core slot structure.
# ---------------------------------------------------------------------------
def _plan(e: np.ndarray):
    """Return (slot_widths, per_core_slots) where per_core_slots[i] is a list
    of (expert_id, sample_indices) aligned with slot_widths (desc order)."""
    order = np.argsort(e, kind="stable")
    counts = np.bincount(e, minlength=E)
    starts = np.concatenate([[0], np.cumsum(counts)])

    chunks = []  # (width, expert, indices)
    for ex in range(E):
        idx = order[starts[ex] : starts[ex + 1]]
        for pos in range(0, len(idx), MAX_W):
            sub = idx[pos : pos + MAX_W]
            chunks.append((len(sub), ex, sub))
    chunks.sort(key=lambda t: -t[0])

    per_core = [[] for _ in range(N_CORES)]
    for r in range(0, len(chunks), N_CORES):
        row = chunks[r : r + N_CORES]
        cores = range(N_CORES) if (r // N_CORES) % 2 == 0 else range(N_CORES - 1, -1, -1)
        for ch, core in zip(row, cores):
            per_core[core].append(ch)

    n_slots = max(len(s) for s in per_core)
    empty = np.zeros((0,), dtype=np.int64)
    for s in per_core:
        while len(s) < n_slots:
            s.append((0, 0, empty))
        s.sort(key=lambda t: -t[0])

    widths = [max(per_core[i][j][0] for i in range(N_CORES)) for j in range(n_slots)]
    widths = [max(w, 1) for w in widths]
    slots = [[(s[j][1], s[j][2]) for j in range(n_slots)] for s in per_core]
    return widths, slots


def _layout(widths, bin_cap=MAX_W):
    """Column layout. Slots are paired; pair p spans widths[2p] columns of
    the packed xT region (slot 2p on partitions 0:64, slot 2p+1 on 64:128).
    Slots are also first-fit packed into "bins" of <=bin_cap y columns;
    each bin is one PSUM bank for the mm2 outputs and one sigmoid
    instruction (bin_cap<=0: one slot per bin, used when b2 varies).
    Returns (pair_offs, NTP, NT, P, S, bins, bin_off, slot_bin, slot_y_off)."""
    S = len(widths)
    P = (S + 1) // 2
    pws = [widths[2 * p] for p in range(P)]
    pair_offs = np.concatenate([[0], np.cumsum(pws)]).astype(np.int64)
    NT = int(np.sum(widths))

    bins = []  # list of [slot indices]
    bin_w = []
    slot_bin = [0] * S
    for j in range(S):
        for b in range(len(bins)):
            if bin_cap > 0 and bin_w[b] + widths[j] <= bin_cap:
                bins[b].append(j)
                bin_w[b] += widths[j]
                slot_bin[j] = b
                break
        else:
            slot_bin[j] = len(bins)
            bins.append([j])
            bin_w.append(widths[j])

    slot_y_off = [0] * S
    off = 0
    bin_off = []
    for b, bslots in enumerate(bins):
        bin_off.append(off)
        for j in bslots:
            slot_y_off[j] = off
            off += widths[j]
    assert off == NT
    return pair_offs, int(pair_offs[-1]), NT, P, S, bins, bin_off, slot_bin, slot_y_off


# ---------------------------------------------------------------------------
# Device graph builder (shared by all cores).
# ---------------------------------------------------------------------------
def _build(widths, b2_uniform):
    pair_offs, NTP, NT, P, S, bins, bin_off, slot_bin, slot_y_off = _layout(
        widths, MAX_W if b2_uniform else 0
    )
    # data tensor columns: [0, P*HID) packed W1 | [P*HID, +S) w2 columns
    # | [W1W2, W1W2+NTP) packed xT
    W2_OFF = P * HID
    XT_OFF = W2_OFF + S
    DCOLS = XT_OFF + NTP
    # first input DMA covers weights + the first SPLIT pairs' xT columns
    SPLIT = min(2, P)
    CUT = int(XT_OFF + pair_offs[SPLIT])
    # output DMA split: y1 covers the first YBINS bins
    YBINS = max(1, len(bins) // 2) if len(bins) > 1 else 0
    YCUT = int(bin_off[YBINS]) if YBINS else 0

    nc = bass.Bass("TRN2", target_bir_lowering=False, debug=False)
    data_e = nc.declare_dram_parameter("data", [128, DCOLS], BF16, isOutput=False)
    bias_e = nc.declare_dram_parameter("bias", [128, 2 * S], F32, isOutput=False)
    y_e = nc.declare_dram_parameter("y", [1, NT], F32, isOutput=True)

    sigmoid = mybir.ActivationFunctionType.Sigmoid
    add = mybir.AluOpType.add
    amax = mybir.AluOpType.max

    NBINS = len(bins)
    with tile.TileContext(nc) as tc:
        with (
            tc.tile_pool(name="sb", bufs=1) as sb,
            tc.tile_pool(name="hp", bufs=4) as hp,
            tc.tile_pool(name="ps1", bufs=3, space="PSUM") as ps1,
            tc.tile_pool(name="ps2", bufs=1, space="PSUM") as ps2,
            tc.tile_pool(name="dummy", bufs=1) as dummy_pool,
        ):
            # ACT sigmoid table preload + PE HAM warmup during the input
            # DMA window: both run on garbage SBUF with no data deps. The
            # warmup matmuls rotate through the same psum bufs the real
            # mm1s use (PE executes in order, so no hazard).
            import os
            WARMUP = os.environ.get("K_WARMUP", "1") == "1"
            if WARMUP:
                warm = dummy_pool.tile([128, 512], BF16)
                warm_in = dummy_pool.tile([1, 16], F32)
                warm_y = dummy_pool.tile([1, 16], F32)
                nc.gpsimd.memset(warm[:], 0.0)
                nc.gpsimd.memset(warm_in[:], 0.0)
                nc.scalar.activation(warm_y[:], warm_in[:], sigmoid)
                for _ in range(5):
                    warm_ps = ps1.tile([HID, 448], F32, tag="p1")
                    nc.tensor.matmul(
                        warm_ps[:], warm[:, :128], warm[:, :448],
                        start=True, stop=True,
                    )

            data1 = sb.tile([128, CUT], BF16)
            data2 = sb.tile([128, DCOLS - CUT], BF16)
            bias = sb.tile([128, 2 * S], F32)
            y1 = sb.tile([1, YCUT], F32)
            y2 = sb.tile([1, NT - YCUT], F32)
            HALF = CUT // 2
            nc.sync.dma_start(data1[:, :HALF], data_e[:, :HALF])
            nc.scalar.dma_start(data1[:, HALF:], data_e[:, HALF:CUT])
            nc.sync.dma_start(data2[:], data_e[:, CUT:])
            nc.scalar.dma_start(bias[:], bias_e[:])

            def dcols(c0, c1, r0=0, r1=128):
                if c1 <= CUT:
                    return data1[r0:r1, c0:c1]
                assert c0 >= CUT
                return data2[r0:r1, c0 - CUT : c1 - CUT]

            def yslice(c0, c1):
                if c1 <= YCUT:
                    return y1[:, c0:c1]
                assert c0 >= YCUT
                return y2[:, c0 - YCUT : c1 - YCUT]

            def slot_aps(j):
                p, hi = divmod(j, 2)
                r0 = 64 * hi
                wj = widths[j]
                c0 = XT_OFF + int(pair_offs[p])
                xt = dcols(c0, c0 + wj, r0, r0 + 64)
                w1 = dcols(p * HID, (p + 1) * HID, r0, r0 + 64)
                return xt, w1

            def mm1(j):
                wj = widths[j]
                xt, w1 = slot_aps(j)
                p1 = ps1.tile([HID, wj], F32, tag="p1")
                nc.tensor.matmul(p1[:], w1, xt, start=True, stop=True)
                return p1

            def relu(j, p1):
                wj = widths[j]
                h = hp.tile([HID, wj], BF16, tag="h")
                nc.vector.tensor_scalar(
                    h[:], p1[:], bias[:, j : j + 1], 0.0, add, amax
                )
                return h

            # one PSUM bank per bin; mm2 of each slot writes its column
            # range, one sigmoid per bin reads the whole bank.
            bin_ps = []
            for b in range(NBINS):
                bw = int(sum(widths[j] for j in bins[b]))
                bin_tile = ps2.tile([1, bw], F32, tag=f"bin{b}")
                bin_ps.append(bin_tile)
            bin_left = [len(bs) for bs in bins]

            def mm2(j, h):
                wj = widths[j]
                b = slot_bin[j]
                c0 = int(slot_y_off[j] - bin_off[b])
                nc.tensor.matmul(
                    bin_ps[b][:, c0 : c0 + wj],
                    dcols(W2_OFF + j, W2_OFF + j + 1), h[:],
                    start=True, stop=True,
                )

            def sig_bin(b):
                c0 = int(bin_off[b])
                wb = bin_ps[b].shape[-1]
                if b2_uniform:
                    bias_ap = bias[0:1, S : S + 1]
                else:
                    bias_ap = bias[0:1, S + bins[b][0] : S + bins[b][0] + 1]
                nc.scalar.activation(
                    yslice(c0, c0 + wb), bin_ps[b][:], sigmoid, bias=bias_ap
                )

            def finish_slot(j):
                b = slot_bin[j]
                bin_left[b] -= 1
                if bin_left[b] == 0:
                    sig_bin(b)
                    if YCUT and all(
                        bin_left[bb] == 0 for bb in range(YBINS)
                    ) and b < YBINS:
                        nc.sync.dma_start(y_e[:, :YCUT], y1[:])

            # software-pipelined emission: mm1 of pair p+1 runs on PE while
            # DVE does relu of pair p; mm2 of pair p follows.
            stage = []  # (j, p1)
            for p in range(P + 1):
                if p < P:
                    js = [2 * p] + ([2 * p + 1] if 2 * p + 1 < S else [])
                    nxt = [(j, mm1(j)) for j in js]
                else:
                    nxt = []
                for j, p1 in stage:
                    h = relu(j, p1)
                    mm2(j, h)
                    finish_slot(j)
                stage = nxt

            if YCUT and any(bin_left[bb] > 0 for bb in range(YBINS)):
                nc.sync.dma_start(y_e[:, :YCUT], y1[:])
            if YCUT:
                nc.sync.dma_start(y_e[:, YCUT:], y2[:])
            else:
                nc.sync.dma_start(y_e[:], y2[:])

    _split_multi_waits(nc)
    return nc


# ---------------------------------------------------------------------------
# Entry point.
# ---------------------------------------------------------------------------
def _run(inputs, trace=False):
    x = np.asarray(inputs["x"], dtype=np.float32)
    num = np.asarray(inputs["num"])
    c = np.asarray(inputs["c"])
    W1 = np.asarray(inputs["W1"], dtype=np.float32)
    b1 = np.asarray(inputs["b1"], dtype=np.float32)
    W2 = np.asarray(inputs["W2"], dtype=np.float32)
    b2 = np.asarray(inputs["b2"], dtype=np.float32)

    B = x.shape[0]
    e = c[num].astype(np.int64)
    b2_uniform = bool(np.all(b2 == b2.flat[0]))
    widths, slots = _plan(e)
    pair_offs, NTP, NT, P, S, bins, bin_off, slot_bin, slot_y_off = _layout(
        widths, MAX_W if b2_uniform else 0
    )
    W2_OFF = P * HID
    XT_OFF = W2_OFF + S
    DCOLS = XT_OFF + NTP

    x_bf = x.astype(NP_BF16)
    W1_bf = W1.astype(NP_BF16)
    W2_bf = W2.astype(NP_BF16)

    in_maps = []
    for core in range(N_CORES):
        data_c = np.zeros((128, DCOLS), dtype=NP_BF16)
        bias_c = np.zeros((128, 2 * S), dtype=np.float32)
        for j in range(S):
            ex, idx = slots[core][j]
            p, hi = divmod(j, 2)
            r0 = 64 * hi
            if len(idx):
                data_c[
                    r0 : r0 + 64,
                    XT_OFF + pair_offs[p] : XT_OFF + pair_offs[p] + len(idx),
                ] = x_bf[idx].T
            data_c[r0 : r0 + 64, p * HID : (p + 1) * HID] = W1_bf[ex]
            data_c[:, W2_OFF + j] = W2_bf[ex, :, 0]
            bias_c[:, j] = b1[ex]
            bias_c[0, S + j] = b2[ex, 0]
        bias_c[0, S] = b2.flat[0] if b2_uniform else bias_c[0, S]
        in_maps.append({"data": data_c, "bias": bias_c})

    nc = _build(widths, b2_uniform)
    res = run_bass_kernel_spmd(nc, in_maps, list(range(N_CORES)), trace=trace)

    out = np.empty((B, 1), dtype=np.float32)
    for core in range(N_CORES):
        y_c = res.results[core]["y"]
        for j in range(S):
            ex, idx = slots[core][j]
            if len(idx):
                out[idx, 0] = y_c[0, slot_y_off[j] : slot_y_off[j] + len(idx)]
    return out, res


def kernel(**inputs) -> np.ndarray:
    out, _ = _run(inputs, trace=False)
    return out


# revision 25
# speedup vs baseline: 1.8198x; 1.0174x over previous
"""MoE routing kernel for Trainium2 (8 NeuronCores).

Reference computation (B=16384, IN=64, HID=128, OUT=1, E=64, NMAP=1000):
    e = c[num]                                  # [B] expert id per sample
    h = relu(x @ W1[e] + b1[e])                 # [B, HID]
    y = sigmoid(h @ W2[e] + b2[e])              # [B, OUT]

Strategy: sort-by-expert dispatch on the host (the routing is pure
integer bookkeeping), dense per-expert matmuls on device. Each core gets
the same static slot structure (SPMD: one graph for all 8 cores); slot
widths are equalized across cores by snake-dealing the per-expert chunks
in descending size order, padding each slot to the max width over cores.

Device data layout (per core): slots are paired onto the 128 SBUF
partitions — pair p puts slot 2p's x^T on partitions 0:64 and slot
2p+1's on 64:128. This gives full-width DMA and lets the two K=64
matmuls of a pair run concurrently in disjoint PE row groups. All
tensor data is bf16 (rel-err budget 2e-2); accumulation stays f32.

Per slot j (width Wj <= 512):
    mm1:   psum1[HID=128, Wj] = W1_j[64,128].T @ xT[64, Wj]     (PE)
    relu:  h[128, Wj] = bf16(max(psum1 + b1_j, 0))              (DVE)
    mm2:   psum2[1, Wj] = w2_j[128,1].T @ h[128, Wj]            (PE)
    sig:   y[1, Wj] = sigmoid(psum2 + b2_j)                     (ACT)
"""

import sys

if "/opt/trn_rl_repo" not in sys.path:
    sys.path.insert(0, "/opt/trn_rl_repo")

import numpy as np

import concourse.bass as bass
import concourse.mybir as mybir
from concourse import tile
from concourse.bass_utils import run_bass_kernel_spmd

N_CORES = 8
IN = 64
HID = 128
E = 64
MAX_W = 512  # moving-operand / PSUM-bank limit

BF16 = mybir.dt.bfloat16
F32 = mybir.dt.float32
NP_BF16 = mybir.dt.np(BF16)


# ---------------------------------------------------------------------------
# This container's walrus build rejects more than one sync wait per
# instruction ("Too many sync wait commands"). Post-pass over the lowered
# BIR: move the extra waits onto single-wait NOPs inserted just before the
# instruction on the same engine (program order makes this equivalent).
# ---------------------------------------------------------------------------
_MAX_WAITS = 1


def _slim_drain_and_barrier(self, tick_clock, wait_clock):
    """Replacement for TileContext._drain_and_barrier: the NEFF here runs
    exactly once per load (run_bass_via_pjrt → single execute), so skip
    the semaphore re-zeroing and the second barrier, and use the
    sequencer-only barrier (no per-engine InstDrain flushes)."""
    drain_inst = self.nc.sync.drain()
    wait_clock.add_sem_waits(
        drain_inst.ins, tile.ScopedClock({None: tick_clock.global_clock})
    )
    popped = self.nc._tile_sem_poison_stack.pop()
    assert popped is self._sem_poison


tile.TileContext._drain_and_barrier = _slim_drain_and_barrier


def _split_multi_waits(nc):
    ctr = 0
    for f in nc.m.functions:
        for blk in f.blocks:
            new_list = []
            for ins in blk.instructions:
                si = ins.sync_info
                if si is not None and si.on_wait and len(si.on_wait) > _MAX_WAITS:
                    waits = list(si.on_wait)
                    head, tail = waits[:-_MAX_WAITS], waits[-_MAX_WAITS:]
                    for i in range(0, len(head), _MAX_WAITS):
                        ctr += 1
                        new_list.append(
                            mybir.InstNoOp(
                                name=f"waitsplit-{ctr}",
                                engine=ins.engine,
                                bass_nofuse=True,
                                sync_info=mybir.SyncInfo(
                                    on_wait=head[i : i + _MAX_WAITS], on_update=[]
                                ),
                            )
                        )
                    si.on_wait = tail
                    ins.sync_info = si
                new_list.append(ins)
            blk.instructions = new_list


# ---------------------------------------------------------------------------
# Host-side routing: build the per-core slot structure.
# ---------------------------------------------------------------------------
def _plan(e: np.ndarray):
    """Return (slot_widths, per_core_slots) where per_core_slots[i] is a list
    of (expert_id, sample_indices) aligned with slot_widths (desc order)."""
    order = np.argsort(e, kind="stable")
    counts = np.bincount(e, minlength=E)
    starts = np.concatenate([[0], np.cumsum(counts)])

    chunks = []  # (width, expert, indices)
    for ex in range(E):
        idx = order[starts[ex] : starts[ex + 1]]
        for pos in range(0, len(idx), MAX_W):
            sub = idx[pos : pos + MAX_W]
            chunks.append((len(sub), ex, sub))
    chunks.sort(key=lambda t: -t[0])

    per_core = [[] for _ in range(N_CORES)]
    for r in range(0, len(chunks), N_CORES):
        row = chunks[r : r + N_CORES]
        cores = range(N_CORES) if (r // N_CORES) % 2 == 0 else range(N_CORES - 1, -1, -1)
        for ch, core in zip(row, cores):
            per_core[core].append(ch)

    n_slots = max(len(s) for s in per_core)
    empty = np.zeros((0,), dtype=np.int64)
    for s in per_core:
        while len(s) < n_slots:
            s.append((0, 0, empty))
        s.sort(key=lambda t: -t[0])

    widths = [max(per_core[i][j][0] for i in range(N_CORES)) for j in range(n_slots)]
    widths = [max(w, 1) for w in widths]
    slots = [[(s[j][1], s[j][2]) for j in range(n_slots)] for s in per_core]
    return widths, slots


def _layout(widths, bin_cap=MAX_W):
    """Column layout. Slots are paired; pair p spans widths[2p] columns of
    the packed xT region (slot 2p on partitions 0:64, slot 2p+1 on 64:128).
    Slots are also first-fit packed into "bins" of <=bin_cap y columns;
    each bin is one PSUM bank for the mm2 outputs and one sigmoid
    instruction (bin_cap<=0: one slot per bin, used when b2 varies).
    Returns (pair_offs, NTP, NT, P, S, bins, bin_off, slot_bin, slot_y_off)."""
    S = len(widths)
    P = (S + 1) // 2
    pws = [widths[2 * p] for p in range(P)]
    pair_offs = np.concatenate([[0], np.cumsum(pws)]).astype(np.int64)
    NT = int(np.sum(widths))

    bins = []  # list of [slot indices]
    bin_w = []
    slot_bin = [0] * S
    for j in range(S):
        for b in range(len(bins)):
            if bin_cap > 0 and bin_w[b] + widths[j] <= bin_cap:
                bins[b].append(j)
                bin_w[b] += widths[j]
                slot_bin[j] = b
                break
        else:
            slot_bin[j] = len(bins)
            bins.append([j])
            bin_w.append(widths[j])

    slot_y_off = [0] * S
    off = 0
    bin_off = []
    for b, bslots in enumerate(bins):
        bin_off.append(off)
        for j in bslots:
            slot_y_off[j] = off
            off += widths[j]
    assert off == NT
    return pair_offs, int(pair_offs[-1]), NT, P, S, bins, bin_off, slot_bin, slot_y_off


# ---------------------------------------------------------------------------
# Device graph builder (shared by all cores).
# ---------------------------------------------------------------------------
def _build(widths, b2_uniform):
    pair_offs, NTP, NT, P, S, bins, bin_off, slot_bin, slot_y_off = _layout(
        widths, MAX_W if b2_uniform else 0
    )
    # data tensor columns: [0, P*HID) packed W1 | [P*HID, +S) w2 columns
    # | [W1W2, W1W2+NTP) packed xT
    W2_OFF = P * HID
    XT_OFF = W2_OFF + S
    DCOLS = XT_OFF + NTP
    # first input DMA covers weights + the first SPLIT pairs' xT columns
    SPLIT = min(2, P)
    CUT = int(XT_OFF + pair_offs[SPLIT])
    # output DMA split: y1 covers the first YBINS bins
    YBINS = max(1, len(bins) // 2) if len(bins) > 1 else 0
    YCUT = int(bin_off[YBINS]) if YBINS else 0

    nc = bass.Bass("TRN2", target_bir_lowering=False, debug=False)
    data_e = nc.declare_dram_parameter("data", [128, DCOLS], BF16, isOutput=False)
    bias_e = nc.declare_dram_parameter("bias", [128, 2 * S], F32, isOutput=False)
    y_e = nc.declare_dram_parameter("y", [1, NT], F32, isOutput=True)

    sigmoid = mybir.ActivationFunctionType.Sigmoid
    add = mybir.AluOpType.add
    amax = mybir.AluOpType.max

    NBINS = len(bins)
    with tile.TileContext(nc) as tc:
        with (
            tc.tile_pool(name="sb", bufs=1) as sb,
            tc.tile_pool(name="hp", bufs=4) as hp,
            tc.tile_pool(name="ps1", bufs=3, space="PSUM") as ps1,
            tc.tile_pool(name="ps2", bufs=1, space="PSUM") as ps2,
            tc.tile_pool(name="dummy", bufs=1) as dummy_pool,
        ):
            # ACT sigmoid table preload + PE HAM warmup during the input
            # DMA window: both run on garbage SBUF with no data deps. The
            # warmup matmuls rotate through the same psum bufs the real
            # mm1s use (PE executes in order, so no hazard).
            import os
            WARMUP = os.environ.get("K_WARMUP", "1") == "1"
            if WARMUP:
                warm = dummy_pool.tile([128, 512], BF16)
                warm_in = dummy_pool.tile([1, 16], F32)
                warm_y = dummy_pool.tile([1, 16], F32)
                nc.gpsimd.memset(warm[:], 0.0)
                nc.gpsimd.memset(warm_in[:], 0.0)
                nc.scalar.activation(warm_y[:], warm_in[:], sigmoid)
                for _ in range(5):
                    warm_ps = ps1.tile([HID, 448], F32, tag="p1")
                    nc.tensor.matmul(
                        warm_ps[:], warm[:, :128], warm[:, :448],
                        start=True, stop=True,
                    )

            data1 = sb.tile([128, CUT], BF16)
            data2 = sb.tile([128, DCOLS - CUT], BF16)
            bias = sb.tile([128, 2 * S], F32)
            y1 = sb.tile([1, YCUT], F32)
            y2 = sb.tile([1, NT - YCUT], F32)
            HALF = CUT // 2
            nc.sync.dma_start(data1[:, :HALF], data_e[:, :HALF])
            nc.scalar.dma_start(data1[:, HALF:], data_e[:, HALF:CUT])
            nc.sync.dma_start(data2[:], data_e[:, CUT:])
            nc.scalar.dma_start(bias[:], bias_e[:])

            def dcols(c0, c1, r0=0, r1=128):
                if c1 <= CUT:
                    return data1[r0:r1, c0:c1]
                assert c0 >= CUT
                return data2[r0:r1, c0 - CUT : c1 - CUT]

            def yslice(c0, c1):
                if c1 <= YCUT:
                    return y1[:, c0:c1]
                assert c0 >= YCUT
                return y2[:, c0 - YCUT : c1 - YCUT]

            def slot_aps(j):
                p, hi = divmod(j, 2)
                r0 = 64 * hi
                wj = widths[j]
                c0 = XT_OFF + int(pair_offs[p])
                xt = dcols(c0, c0 + wj, r0, r0 + 64)
                w1 = dcols(p * HID, (p + 1) * HID, r0, r0 + 64)
                return xt, w1

            def mm1(j):
                wj = widths[j]
                xt, w1 = slot_aps(j)
                p1 = ps1.tile([HID, wj], F32, tag="p1")
                nc.tensor.matmul(p1[:], w1, xt, start=True, stop=True)
                return p1

            def relu(j, p1):
                wj = widths[j]
                h = hp.tile([HID, wj], BF16, tag="h")
                nc.vector.tensor_scalar(
                    h[:], p1[:], bias[:, j : j + 1], 0.0, add, amax
                )
                return h

            # one PSUM bank per bin; mm2 of each slot writes its column
            # range, one sigmoid per bin reads the whole bank.
            bin_ps = []
            for b in range(NBINS):
                bw = int(sum(widths[j] for j in bins[b]))
                bin_tile = ps2.tile([1, bw], F32, tag=f"bin{b}")
                bin_ps.append(bin_tile)
            bin_left = [len(bs) for bs in bins]

            def mm2(j, h):
                wj = widths[j]
                b = slot_bin[j]
                c0 = int(slot_y_off[j] - bin_off[b])
                nc.tensor.matmul(
                    bin_ps[b][:, c0 : c0 + wj],
                    dcols(W2_OFF + j, W2_OFF + j + 1), h[:],
                    start=True, stop=True,
                )

            def sig_bin(b):
                c0 = int(bin_off[b])
                wb = bin_ps[b].shape[-1]
                if b2_uniform:
                    bias_ap = bias[0:1, S : S + 1]
                else:
                    bias_ap = bias[0:1, S + bins[b][0] : S + bins[b][0] + 1]
                nc.scalar.activation(
                    yslice(c0, c0 + wb), bin_ps[b][:], sigmoid, bias=bias_ap
                )

            def finish_slot(j):
                b = slot_bin[j]
                bin_left[b] -= 1
                if bin_left[b] == 0:
                    sig_bin(b)
                    if YCUT and all(
                        bin_left[bb] == 0 for bb in range(YBINS)
                    ) and b < YBINS:
                        nc.sync.dma_start(y_e[:, :YCUT], y1[:])

            # software-pipelined emission: mm1 of pair p+1 runs on PE while
            # DVE does relu of pair p; mm2 of pair p follows.
            stage = []  # (j, p1)
            for p in range(P + 1):
                if p < P:
                    js = [2 * p] + ([2 * p + 1] if 2 * p + 1 < S else [])
                    nxt = [(j, mm1(j)) for j in js]
                else:
                    nxt = []
                for j, p1 in stage:
                    h = relu(j, p1)
                    mm2(j, h)
                    finish_slot(j)
                stage = nxt

            if YCUT and any(bin_left[bb] > 0 for bb in range(YBINS)):
                nc.sync.dma_start(y_e[:, :YCUT], y1[:])
            if YCUT:
                nc.sync.dma_start(y_e[:, YCUT:], y2[:])
            else:
                nc.sync.dma_start(y_e[:], y2[:])

    _split_multi_waits(nc)
    return nc


# ---------------------------------------------------------------------------
# Entry point.
# ---------------------------------------------------------------------------
def _run(inputs, trace=False):
    x = np.asarray(inputs["x"], dtype=np.float32)
    num = np.asarray(inputs["num"])
    c = np.asarray(inputs["c"])
    W1 = np.asarray(inputs["W1"], dtype=np.float32)
    b1 = np.asarray(inputs["b1"], dtype=np.float32)
    W2 = np.asarray(inputs["W2"], dtype=np.float32)
    b2 = np.asarray(inputs["b2"], dtype=np.float32)

    B = x.shape[0]
    e = c[num].astype(np.int64)
    b2_uniform = bool(np.all(b2 == b2.flat[0]))
    widths, slots = _plan(e)
    pair_offs, NTP, NT, P, S, bins, bin_off, slot_bin, slot_y_off = _layout(
        widths, MAX_W if b2_uniform else 0
    )
    W2_OFF = P * HID
    XT_OFF = W2_OFF + S
    DCOLS = XT_OFF + NTP

    x_bf = x.astype(NP_BF16)
    W1_bf = W1.astype(NP_BF16)
    W2_bf = W2.astype(NP_BF16)

    in_maps = []
    for core in range(N_CORES):
        data_c = np.zeros((128, DCOLS), dtype=NP_BF16)
        bias_c = np.zeros((128, 2 * S), dtype=np.float32)
        for j in range(S):
            ex, idx = slots[core][j]
            p, hi = divmod(j, 2)
            r0 = 64 * hi
            if len(idx):
                data_c[
                    r0 : r0 + 64,
                    XT_OFF + pair_offs[p] : XT_OFF + pair_offs[p] + len(idx),
                ] = x_bf[idx].T
            data_c[r0 : r0 + 64, p * HID : (p + 1) * HID] = W1_bf[ex]
            data_c[:, W2_OFF + j] = W2_bf[ex, :, 0]
            bias_c[:, j] = b1[ex]
            bias_c[0, S + j] = b2[ex, 0]
        bias_c[0, S] = b2.flat[0] if b2_uniform else bias_c[0, S]
        in_maps.append({"data": data_c, "bias": bias_c})

    nc = _build(widths, b2_uniform)
    res = run_bass_kernel_spmd(nc, in_maps, list(range(N_CORES)), trace=trace)

    out = np.empty((B, 1), dtype=np.float32)
    for core in range(N_CORES):
        y_c = res.results[core]["y"]
        for j in range(S):
            ex, idx = slots[core][j]
            if len(idx):
                out[idx, 0] = y_c[0, slot_y_off[j] : slot_y_off[j] + len(idx)]
    return out, res


def kernel(**inputs) -> np.ndarray:
    out, _ = _run(inputs, trace=False)
    return out


# revision 32
# speedup vs baseline: 1.8217x; 1.0010x over previous
"""MoE routing kernel for Trainium2 (8 NeuronCores).

Reference computation (B=16384, IN=64, HID=128, OUT=1, E=64, NMAP=1000):
    e = c[num]                                  # [B] expert id per sample
    h = relu(x @ W1[e] + b1[e])                 # [B, HID]
    y = sigmoid(h @ W2[e] + b2[e])              # [B, OUT]

Strategy: sort-by-expert dispatch on the host (the routing is pure
integer bookkeeping), dense per-expert matmuls on device. Each core gets
the same static slot structure (SPMD: one graph for all 8 cores); slot
widths are equalized across cores by snake-dealing the per-expert chunks
in descending size order, padding each slot to the max width over cores.

Device data layout (per core): slots are paired onto the 128 SBUF
partitions — pair p puts slot 2p's x^T on partitions 0:64 and slot
2p+1's on 64:128. This gives full-width DMA and lets the two K=64
matmuls of a pair run concurrently in disjoint PE row groups. All
tensor data is bf16 (rel-err budget 2e-2); accumulation stays f32.

Per slot j (width Wj <= 512):
    mm1:   psum1[HID=128, Wj] = W1_j[64,128].T @ xT[64, Wj]     (PE)
    relu:  h[128, Wj] = bf16(max(psum1 + b1_j, 0))              (DVE)
    mm2:   psum2[1, Wj] = w2_j[128,1].T @ h[128, Wj]            (PE)
    sig:   y[1, Wj] = sigmoid(psum2 + b2_j)                     (ACT)
"""

import sys

if "/opt/trn_rl_repo" not in sys.path:
    sys.path.insert(0, "/opt/trn_rl_repo")

import numpy as np

import concourse.bass as bass
import concourse.mybir as mybir
from concourse import tile
from concourse.bass_utils import run_bass_kernel_spmd

N_CORES = 8
IN = 64
HID = 128
E = 64
MAX_W = 512  # moving-operand / PSUM-bank limit

BF16 = mybir.dt.bfloat16
F32 = mybir.dt.float32
NP_BF16 = mybir.dt.np(BF16)


# ---------------------------------------------------------------------------
# This container's walrus build rejects more than one sync wait per
# instruction ("Too many sync wait commands"). Post-pass over the lowered
# BIR: move the extra waits onto single-wait NOPs inserted just before the
# instruction on the same engine (program order makes this equivalent).
# ---------------------------------------------------------------------------
_MAX_WAITS = 1


def _slim_drain_and_barrier(self, tick_clock, wait_clock):
    """Replacement for TileContext._drain_and_barrier: the NEFF here runs
    exactly once per load (run_bass_via_pjrt → single execute), so skip
    the semaphore re-zeroing and the second barrier, and use the
    sequencer-only barrier (no per-engine InstDrain flushes)."""
    drain_inst = self.nc.sync.drain()
    wait_clock.add_sem_waits(
        drain_inst.ins, tile.ScopedClock({None: tick_clock.global_clock})
    )
    popped = self.nc._tile_sem_poison_stack.pop()
    assert popped is self._sem_poison


tile.TileContext._drain_and_barrier = _slim_drain_and_barrier


def _filter_drain_waits(nc, out_dma_names):
    """The kernel-tail drain only needs to gate on the output DMAs'
    completion semaphores — every other wait Tile put on it is
    transitively implied. Fewer waits = fewer single-wait NOPs."""
    keep_ids = set()
    drain = None
    for f in nc.m.functions:
        for blk in f.blocks:
            for ins in blk.instructions:
                if ins.name in out_dma_names and ins.sync_info is not None:
                    for u in ins.sync_info.on_update:
                        keep_ids.add(u.id)
                if isinstance(ins, mybir.InstDrain):
                    si = ins.sync_info
                    if si is not None and len(si.on_wait) > 1:
                        drain = ins
    if drain is None or not keep_ids:
        return
    si = drain.sync_info
    kept = [w for w in si.on_wait if w.id in keep_ids]
    if kept:
        si.on_wait = kept
        drain.sync_info = si


def _split_multi_waits(nc):
    ctr = 0
    for f in nc.m.functions:
        for blk in f.blocks:
            new_list = []
            for ins in blk.instructions:
                si = ins.sync_info
                if si is not None and si.on_wait and len(si.on_wait) > _MAX_WAITS:
                    waits = list(si.on_wait)
                    head, tail = waits[:-_MAX_WAITS], waits[-_MAX_WAITS:]
                    for i in range(0, len(head), _MAX_WAITS):
                        ctr += 1
                        new_list.append(
                            mybir.InstNoOp(
                                name=f"waitsplit-{ctr}",
                                engine=ins.engine,
                                bass_nofuse=True,
                                sync_info=mybir.SyncInfo(
                                    on_wait=head[i : i + _MAX_WAITS], on_update=[]
                                ),
                            )
                        )
                    si.on_wait = tail
                    ins.sync_info = si
                new_list.append(ins)
            blk.instructions = new_list


# ---------------------------------------------------------------------------
# Host-side routing: build the per-core slot structure.
# ---------------------------------------------------------------------------
def _plan(e: np.ndarray):
    """Return (slot_widths, per_core_slots) where per_core_slots[i] is a list
    of (expert_id, sample_indices) aligned with slot_widths (desc order)."""
    order = np.argsort(e, kind="stable")
    counts = np.bincount(e, minlength=E)
    starts = np.concatenate([[0], np.cumsum(counts)])

    chunks = []  # (width, expert, indices)
    for ex in range(E):
        idx = order[starts[ex] : starts[ex + 1]]
        for pos in range(0, len(idx), MAX_W):
            sub = idx[pos : pos + MAX_W]
            chunks.append((len(sub), ex, sub))
    chunks.sort(key=lambda t: -t[0])

    per_core = [[] for _ in range(N_CORES)]
    for r in range(0, len(chunks), N_CORES):
        row = chunks[r : r + N_CORES]
        cores = range(N_CORES) if (r // N_CORES) % 2 == 0 else range(N_CORES - 1, -1, -1)
        for ch, core in zip(row, cores):
            per_core[core].append(ch)

    n_slots = max(len(s) for s in per_core)
    empty = np.zeros((0,), dtype=np.int64)
    for s in per_core:
        while len(s) < n_slots:
            s.append((0, 0, empty))
        s.sort(key=lambda t: -t[0])

    widths = [max(per_core[i][j][0] for i in range(N_CORES)) for j in range(n_slots)]
    widths = [max(w, 1) for w in widths]
    slots = [[(s[j][1], s[j][2]) for j in range(n_slots)] for s in per_core]
    return widths, slots


def _layout(widths, bin_cap=MAX_W):
    """Column layout. Slots are paired; pair p spans widths[2p] columns of
    the packed xT region (slot 2p on partitions 0:64, slot 2p+1 on 64:128).
    Slots are also first-fit packed into "bins" of <=bin_cap y columns;
    each bin is one PSUM bank for the mm2 outputs and one sigmoid
    instruction (bin_cap<=0: one slot per bin, used when b2 varies).
    Returns (pair_offs, NTP, NT, P, S, bins, bin_off, slot_bin, slot_y_off)."""
    S = len(widths)
    P = (S + 1) // 2
    # pair widths rounded to even so bf16 column cuts stay 4B-aligned
    # (the bias region is bitcast to f32)
    pws = [widths[2 * p] + (widths[2 * p] & 1) for p in range(P)]
    pair_offs = np.concatenate([[0], np.cumsum(pws)]).astype(np.int64)
    NT = int(np.sum(widths))

    bins = []  # list of [slot indices]
    bin_w = []
    slot_bin = [0] * S
    for j in range(S):
        for b in range(len(bins)):
            if bin_cap > 0 and bin_w[b] + widths[j] <= bin_cap:
                bins[b].append(j)
                bin_w[b] += widths[j]
                slot_bin[j] = b
                break
        else:
            slot_bin[j] = len(bins)
            bins.append([j])
            bin_w.append(widths[j])

    slot_y_off = [0] * S
    off = 0
    bin_off = []
    for b, bslots in enumerate(bins):
        bin_off.append(off)
        for j in bslots:
            slot_y_off[j] = off
            off += widths[j]
    assert off == NT
    return pair_offs, int(pair_offs[-1]), NT, P, S, bins, bin_off, slot_bin, slot_y_off


# ---------------------------------------------------------------------------
# Device graph builder (shared by all cores).
# ---------------------------------------------------------------------------
def _build(widths, b2_uniform):
    pair_offs, NTP, NT, P, S, bins, bin_off, slot_bin, slot_y_off = _layout(
        widths, MAX_W if b2_uniform else 0
    )
    # data tensor columns (bf16 units): [0, 4S) f32 biases bitcast (b1
    # cols then b2 row) | [4S, +P*HID) packed W1 | [.., +S) w2 columns
    # | [XT_OFF, +NTP) packed xT
    B_OFF = 0
    W1_OFF = 4 * S
    W2_OFF = W1_OFF + P * HID
    XT_OFF = W2_OFF + S
    DCOLS = XT_OFF + NTP
    # input DMA split at pair boundaries: A = bias+weights+pair0 (sync),
    # B = pair1 (scalar), C = pairs 2.. (sync)
    CUT1 = int(XT_OFF + pair_offs[min(1, P)])
    CUT2 = int(XT_OFF + pair_offs[min(2, P)])
    # output DMA split: y1 covers the first YBINS bins
    YBINS = max(1, len(bins) // 2) if len(bins) > 1 else 0
    YCUT = int(bin_off[YBINS]) if YBINS else 0

    nc = bass.Bass("TRN2", target_bir_lowering=False, debug=False)
    data_e = nc.declare_dram_parameter("data", [128, DCOLS], BF16, isOutput=False)
    y_e = nc.declare_dram_parameter("y", [1, NT], F32, isOutput=True)

    sigmoid = mybir.ActivationFunctionType.Sigmoid
    add = mybir.AluOpType.add
    amax = mybir.AluOpType.max

    NBINS = len(bins)
    out_dma_names = []
    with tile.TileContext(nc) as tc:
        with (
            tc.tile_pool(name="sb", bufs=1) as sb,
            tc.tile_pool(name="hp", bufs=4) as hp,
            tc.tile_pool(name="ps1", bufs=3, space="PSUM") as ps1,
            tc.tile_pool(name="ps2", bufs=1, space="PSUM") as ps2,
            tc.tile_pool(name="dummy", bufs=1) as dummy_pool,
        ):
            # Engine preloads during the input DMA window (all on garbage
            # SBUF, no data deps): ACT sigmoid table load, DVE first-op
            # cost, PE pipeline priming. The warmup matmuls rotate through
            # the same psum bufs the real mm1s use (PE executes in order).
            import os
            WARMUP = os.environ.get("K_WARMUP", "1") == "1"
            if WARMUP:
                warm = dummy_pool.tile([128, 512], BF16)
                warm_in = dummy_pool.tile([1, 16], F32)
                warm_y = dummy_pool.tile([1, 16], F32)
                warm_v = dummy_pool.tile([1, 16], F32)
                nc.gpsimd.memset(warm[:], 0.0)
                nc.gpsimd.memset(warm_in[:], 0.0)
                nc.scalar.activation(warm_y[:], warm_in[:], sigmoid)
                nc.vector.tensor_scalar(
                    warm_v[:], warm_in[:], 0.0, 0.0, add, amax
                )
                for _ in range(4):
                    warm_ps = ps1.tile([HID, 448], F32, tag="p1")
                    nc.tensor.matmul(
                        warm_ps[:], warm[:, :128], warm[:, :448],
                        start=True, stop=True,
                    )

            dataA = sb.tile([128, CUT1], BF16)
            dataB = sb.tile([128, max(CUT2 - CUT1, 1)], BF16)
            dataC = sb.tile([128, max(DCOLS - CUT2, 1)], BF16)
            y1 = sb.tile([1, max(YCUT, 1)], F32)
            y2 = sb.tile([1, NT - YCUT], F32)
            nc.sync.dma_start(dataA[:], data_e[:, :CUT1])
            if CUT2 > CUT1:
                nc.scalar.dma_start(dataB[:], data_e[:, CUT1:CUT2])
            if DCOLS > CUT2:
                nc.sync.dma_start(dataC[:], data_e[:, CUT2:])

            def dcols(c0, c1, r0=0, r1=128):
                if c1 <= CUT1:
                    return dataA[r0:r1, c0:c1]
                if c1 <= CUT2:
                    assert c0 >= CUT1
                    return dataB[r0:r1, c0 - CUT1 : c1 - CUT1]
                assert c0 >= CUT2
                return dataC[r0:r1, c0 - CUT2 : c1 - CUT2]

            def b1_ap(j):
                return dataA[:, 2 * j : 2 * j + 2].bitcast(F32)

            def b2_ap(j):
                c = 2 * S + 2 * j
                return dataA[0:1, c : c + 2].bitcast(F32)

            def yslice(c0, c1):
                if c1 <= YCUT:
                    return y1[:, c0:c1]
                assert c0 >= YCUT
                return y2[:, c0 - YCUT : c1 - YCUT]

            def slot_aps(j):
                p, hi = divmod(j, 2)
                r0 = 64 * hi
                wj = widths[j]
                c0 = XT_OFF + int(pair_offs[p])
                xt = dcols(c0, c0 + wj, r0, r0 + 64)
                w1 = dcols(
                    W1_OFF + p * HID, W1_OFF + (p + 1) * HID, r0, r0 + 64
                )
                return xt, w1

            def mm1(j):
                wj = widths[j]
                xt, w1 = slot_aps(j)
                p1 = ps1.tile([HID, wj], F32, tag="p1")
                nc.tensor.matmul(p1[:], w1, xt, start=True, stop=True)
                return p1

            def relu(j, p1):
                wj = widths[j]
                h = hp.tile([HID, wj], BF16, tag="h")
                nc.vector.tensor_scalar(
                    h[:], p1[:], b1_ap(j), 0.0, add, amax
                )
                return h

            # one PSUM bank per bin; mm2 of each slot writes its column
            # range, one sigmoid per bin reads the whole bank.
            bin_ps = []
            for b in range(NBINS):
                bw = int(sum(widths[j] for j in bins[b]))
                bin_tile = ps2.tile([1, bw], F32, tag=f"bin{b}")
                bin_ps.append(bin_tile)
            bin_left = [len(bs) for bs in bins]

            def mm2(j, h):
                wj = widths[j]
                b = slot_bin[j]
                c0 = int(slot_y_off[j] - bin_off[b])
                nc.tensor.matmul(
                    bin_ps[b][:, c0 : c0 + wj],
                    dcols(W2_OFF + j, W2_OFF + j + 1), h[:],
                    start=True, stop=True,
                )

            def sig_bin(b):
                c0 = int(bin_off[b])
                wb = bin_ps[b].shape[-1]
                bias_ap = b2_ap(0 if b2_uniform else bins[b][0])
                nc.scalar.activation(
                    yslice(c0, c0 + wb), bin_ps[b][:], sigmoid, bias=bias_ap
                )

            def finish_slot(j):
                b = slot_bin[j]
                bin_left[b] -= 1
                if bin_left[b] == 0:
                    sig_bin(b)
                    if YCUT and all(
                        bin_left[bb] == 0 for bb in range(YBINS)
                    ) and b < YBINS:
                        d = nc.sync.dma_start(y_e[:, :YCUT], y1[:])
                        out_dma_names.append(d.ins.name)

            # software-pipelined emission: mm1 of pair p+1 runs on PE while
            # DVE does relu of pair p; mm2 of pair p follows.
            stage = []  # (j, p1)
            for p in range(P + 1):
                if p < P:
                    js = [2 * p] + ([2 * p + 1] if 2 * p + 1 < S else [])
                    nxt = [(j, mm1(j)) for j in js]
                else:
                    nxt = []
                for j, p1 in stage:
                    h = relu(j, p1)
                    mm2(j, h)
                    finish_slot(j)
                stage = nxt

            if YCUT and any(bin_left[bb] > 0 for bb in range(YBINS)):
                d = nc.sync.dma_start(y_e[:, :YCUT], y1[:])
                out_dma_names.append(d.ins.name)
            if YCUT:
                d = nc.sync.dma_start(y_e[:, YCUT:], y2[:])
            else:
                d = nc.sync.dma_start(y_e[:], y2[:])
            out_dma_names.append(d.ins.name)

    _filter_drain_waits(nc, out_dma_names)
    _split_multi_waits(nc)
    return nc


# ---------------------------------------------------------------------------
# Entry point.
# ---------------------------------------------------------------------------
def _run(inputs, trace=False):
    x = np.asarray(inputs["x"], dtype=np.float32)
    num = np.asarray(inputs["num"])
    c = np.asarray(inputs["c"])
    W1 = np.asarray(inputs["W1"], dtype=np.float32)
    b1 = np.asarray(inputs["b1"], dtype=np.float32)
    W2 = np.asarray(inputs["W2"], dtype=np.float32)
    b2 = np.asarray(inputs["b2"], dtype=np.float32)

    B = x.shape[0]
    e = c[num].astype(np.int64)
    b2_uniform = bool(np.all(b2 == b2.flat[0]))
    widths, slots = _plan(e)
    pair_offs, NTP, NT, P, S, bins, bin_off, slot_bin, slot_y_off = _layout(
        widths, MAX_W if b2_uniform else 0
    )
    W1_OFF = 4 * S
    W2_OFF = W1_OFF + P * HID
    XT_OFF = W2_OFF + S
    DCOLS = XT_OFF + NTP

    x_bf = x.astype(NP_BF16)
    W1_bf = W1.astype(NP_BF16)
    W2_bf = W2.astype(NP_BF16)

    in_maps = []
    for core in range(N_CORES):
        data_c = np.zeros((128, DCOLS), dtype=NP_BF16)
        bias_c = np.zeros((128, 2 * S), dtype=np.float32)
        for j in range(S):
            ex, idx = slots[core][j]
            p, hi = divmod(j, 2)
            r0 = 64 * hi
            if len(idx):
                data_c[
                    r0 : r0 + 64,
                    XT_OFF + pair_offs[p] : XT_OFF + pair_offs[p] + len(idx),
                ] = x_bf[idx].T
            data_c[r0 : r0 + 64, W1_OFF + p * HID : W1_OFF + (p + 1) * HID] = (
                W1_bf[ex]
            )
            data_c[:, W2_OFF + j] = W2_bf[ex, :, 0]
            bias_c[:, j] = b1[ex]
            bias_c[0, S + j] = b2[ex, 0]
        data_c[:, : 4 * S] = bias_c.view(NP_BF16)
        in_maps.append({"data": data_c})

    nc = _build(widths, b2_uniform)
    res = run_bass_kernel_spmd(nc, in_maps, list(range(N_CORES)), trace=trace)

    out = np.empty((B, 1), dtype=np.float32)
    for core in range(N_CORES):
        y_c = res.results[core]["y"]
        for j in range(S):
            ex, idx = slots[core][j]
            if len(idx):
                out[idx, 0] = y_c[0, slot_y_off[j] : slot_y_off[j] + len(idx)]
    return out, res


def kernel(**inputs) -> np.ndarray:
    out, _ = _run(inputs, trace=False)
    return out


# revision 33
# speedup vs baseline: 1.8260x; 1.0024x over previous
"""MoE routing kernel for Trainium2 (8 NeuronCores).

Reference computation (B=16384, IN=64, HID=128, OUT=1, E=64, NMAP=1000):
    e = c[num]                                  # [B] expert id per sample
    h = relu(x @ W1[e] + b1[e])                 # [B, HID]
    y = sigmoid(h @ W2[e] + b2[e])              # [B, OUT]

Strategy: sort-by-expert dispatch on the host (the routing is pure
integer bookkeeping), dense per-expert matmuls on device. Each core gets
the same static slot structure (SPMD: one graph for all 8 cores); slot
widths are equalized across cores by snake-dealing the per-expert chunks
in descending size order, padding each slot to the max width over cores.

Device data layout (per core): slots are paired onto the 128 SBUF
partitions — pair p puts slot 2p's x^T on partitions 0:64 and slot
2p+1's on 64:128. This gives full-width DMA and lets the two K=64
matmuls of a pair run concurrently in disjoint PE row groups. All
tensor data is bf16 (rel-err budget 2e-2); accumulation stays f32.

Per slot j (width Wj <= 512):
    mm1:   psum1[HID=128, Wj] = W1_j[64,128].T @ xT[64, Wj]     (PE)
    relu:  h[128, Wj] = bf16(max(psum1 + b1_j, 0))              (DVE)
    mm2:   psum2[1, Wj] = w2_j[128,1].T @ h[128, Wj]            (PE)
    sig:   y[1, Wj] = sigmoid(psum2 + b2_j)                     (ACT)
"""

import sys

if "/opt/trn_rl_repo" not in sys.path:
    sys.path.insert(0, "/opt/trn_rl_repo")

import numpy as np

import concourse.bass as bass
import concourse.mybir as mybir
from concourse import tile
from concourse.bass_utils import run_bass_kernel_spmd

N_CORES = 8
IN = 64
HID = 128
E = 64
MAX_W = 512  # moving-operand / PSUM-bank limit

BF16 = mybir.dt.bfloat16
F32 = mybir.dt.float32
NP_BF16 = mybir.dt.np(BF16)


# ---------------------------------------------------------------------------
# This container's walrus build rejects more than one sync wait per
# instruction ("Too many sync wait commands"). Post-pass over the lowered
# BIR: move the extra waits onto single-wait NOPs inserted just before the
# instruction on the same engine (program order makes this equivalent).
# ---------------------------------------------------------------------------
_MAX_WAITS = 1


def _slim_drain_and_barrier(self, tick_clock, wait_clock):
    """Replacement for TileContext._drain_and_barrier: the NEFF here runs
    exactly once per load (run_bass_via_pjrt → single execute), so skip
    the semaphore re-zeroing and the second barrier, and use the
    sequencer-only barrier (no per-engine InstDrain flushes)."""
    drain_inst = self.nc.sync.drain()
    wait_clock.add_sem_waits(
        drain_inst.ins, tile.ScopedClock({None: tick_clock.global_clock})
    )
    popped = self.nc._tile_sem_poison_stack.pop()
    assert popped is self._sem_poison


tile.TileContext._drain_and_barrier = _slim_drain_and_barrier


def _filter_drain_waits(nc, out_dma_names):
    """The kernel-tail drain only needs to gate on the output DMAs'
    completion semaphores — every other wait Tile put on it is
    transitively implied. Fewer waits = fewer single-wait NOPs."""
    keep_ids = set()
    drain = None
    for f in nc.m.functions:
        for blk in f.blocks:
            for ins in blk.instructions:
                if ins.name in out_dma_names and ins.sync_info is not None:
                    for u in ins.sync_info.on_update:
                        keep_ids.add(u.id)
                if isinstance(ins, mybir.InstDrain):
                    si = ins.sync_info
                    if si is not None and len(si.on_wait) > 1:
                        drain = ins
    if drain is None or not keep_ids:
        return
    si = drain.sync_info
    kept = [w for w in si.on_wait if w.id in keep_ids]
    if kept:
        si.on_wait = kept
        drain.sync_info = si


def _split_multi_waits(nc):
    ctr = 0
    for f in nc.m.functions:
        for blk in f.blocks:
            new_list = []
            for ins in blk.instructions:
                si = ins.sync_info
                if si is not None and si.on_wait and len(si.on_wait) > _MAX_WAITS:
                    waits = list(si.on_wait)
                    head, tail = waits[:-_MAX_WAITS], waits[-_MAX_WAITS:]
                    for i in range(0, len(head), _MAX_WAITS):
                        ctr += 1
                        new_list.append(
                            mybir.InstNoOp(
                                name=f"waitsplit-{ctr}",
                                engine=ins.engine,
                                bass_nofuse=True,
                                sync_info=mybir.SyncInfo(
                                    on_wait=head[i : i + _MAX_WAITS], on_update=[]
                                ),
                            )
                        )
                    si.on_wait = tail
                    ins.sync_info = si
                new_list.append(ins)
            blk.instructions = new_list


# ---------------------------------------------------------------------------
# Host-side routing: build the per-core slot structure.
# ---------------------------------------------------------------------------
def _plan(e: np.ndarray):
    """Return (slot_widths, per_core_slots) where per_core_slots[i] is a list
    of (expert_id, sample_indices) aligned with slot_widths (desc order)."""
    order = np.argsort(e, kind="stable")
    counts = np.bincount(e, minlength=E)
    starts = np.concatenate([[0], np.cumsum(counts)])

    chunks = []  # (width, expert, indices)
    for ex in range(E):
        idx = order[starts[ex] : starts[ex + 1]]
        for pos in range(0, len(idx), MAX_W):
            sub = idx[pos : pos + MAX_W]
            chunks.append((len(sub), ex, sub))
    chunks.sort(key=lambda t: -t[0])

    per_core = [[] for _ in range(N_CORES)]
    for r in range(0, len(chunks), N_CORES):
        row = chunks[r : r + N_CORES]
        cores = range(N_CORES) if (r // N_CORES) % 2 == 0 else range(N_CORES - 1, -1, -1)
        for ch, core in zip(row, cores):
            per_core[core].append(ch)

    n_slots = max(len(s) for s in per_core)
    empty = np.zeros((0,), dtype=np.int64)
    for s in per_core:
        while len(s) < n_slots:
            s.append((0, 0, empty))
        s.sort(key=lambda t: -t[0])

    widths = [max(per_core[i][j][0] for i in range(N_CORES)) for j in range(n_slots)]
    widths = [max(w, 1) for w in widths]
    slots = [[(s[j][1], s[j][2]) for j in range(n_slots)] for s in per_core]
    return widths, slots


def _layout(widths, bin_cap=MAX_W):
    """Column layout. Slots are paired; pair p spans widths[2p] columns of
    the packed xT region (slot 2p on partitions 0:64, slot 2p+1 on 64:128).
    Slots are also first-fit packed into "bins" of <=bin_cap y columns;
    each bin is one PSUM bank for the mm2 outputs and one sigmoid
    instruction (bin_cap<=0: one slot per bin, used when b2 varies).
    Returns (pair_offs, NTP, NT, P, S, bins, bin_off, slot_bin, slot_y_off)."""
    S = len(widths)
    P = (S + 1) // 2
    # pair widths rounded to even so bf16 column cuts stay 4B-aligned
    # (the bias region is bitcast to f32)
    pws = [widths[2 * p] + (widths[2 * p] & 1) for p in range(P)]
    pair_offs = np.concatenate([[0], np.cumsum(pws)]).astype(np.int64)
    NT = int(np.sum(widths))

    bins = []  # list of [slot indices]
    bin_w = []
    slot_bin = [0] * S
    for j in range(S):
        for b in range(len(bins)):
            if bin_cap > 0 and bin_w[b] + widths[j] <= bin_cap:
                bins[b].append(j)
                bin_w[b] += widths[j]
                slot_bin[j] = b
                break
        else:
            slot_bin[j] = len(bins)
            bins.append([j])
            bin_w.append(widths[j])

    slot_y_off = [0] * S
    off = 0
    bin_off = []
    for b, bslots in enumerate(bins):
        bin_off.append(off)
        for j in bslots:
            slot_y_off[j] = off
            off += widths[j]
    assert off == NT
    return pair_offs, int(pair_offs[-1]), NT, P, S, bins, bin_off, slot_bin, slot_y_off


# ---------------------------------------------------------------------------
# Device graph builder (shared by all cores).
# ---------------------------------------------------------------------------
def _build(widths, b2_uniform):
    pair_offs, NTP, NT, P, S, bins, bin_off, slot_bin, slot_y_off = _layout(
        widths, MAX_W if b2_uniform else 0
    )
    # data tensor columns (bf16 units): [0, 4S) f32 biases bitcast (b1
    # cols then b2 row) | [4S, +P*HID) packed W1 | [.., +S) w2 columns
    # | [XT_OFF, +NTP) packed xT
    B_OFF = 0
    W1_OFF = 4 * S
    W2_OFF = W1_OFF + P * HID
    XT_OFF = W2_OFF + S
    DCOLS = XT_OFF + NTP
    # input DMA split at pair boundaries: A = bias+weights+pair0 (sync),
    # B = pair1 (scalar), C = pairs 2.. (sync)
    CUT1 = int(XT_OFF + pair_offs[min(1, P)])
    CUT2 = int(XT_OFF + pair_offs[min(2, P)])
    # output DMA split: y1 covers the first YBINS bins
    YBINS = max(1, len(bins) // 2) if len(bins) > 1 else 0
    YCUT = int(bin_off[YBINS]) if YBINS else 0

    nc = bass.Bass("TRN2", target_bir_lowering=False, debug=False)
    data_e = nc.declare_dram_parameter("data", [128, DCOLS], BF16, isOutput=False)
    y_e = nc.declare_dram_parameter("y", [1, NT], F32, isOutput=True)

    sigmoid = mybir.ActivationFunctionType.Sigmoid
    add = mybir.AluOpType.add
    amax = mybir.AluOpType.max

    NBINS = len(bins)
    out_dma_names = []
    with tile.TileContext(nc) as tc:
        with (
            tc.tile_pool(name="sb", bufs=1) as sb,
            tc.tile_pool(name="hp", bufs=4) as hp,
            tc.tile_pool(name="ps1", bufs=3, space="PSUM") as ps1,
            tc.tile_pool(name="ps2", bufs=1, space="PSUM") as ps2,
            tc.tile_pool(name="dummy", bufs=1) as dummy_pool,
        ):
            # Engine preloads during the input DMA window (all on garbage
            # SBUF, no data deps): ACT sigmoid table load, DVE first-op
            # cost, PE pipeline priming. The warmup matmuls rotate through
            # the same psum bufs the real mm1s use (PE executes in order).
            import os
            WARMUP = os.environ.get("K_WARMUP", "1") == "1"
            if WARMUP:
                warm = dummy_pool.tile([128, 512], BF16)
                warm_in = dummy_pool.tile([1, 16], F32)
                warm_y = dummy_pool.tile([1, 16], F32)
                warm_v = dummy_pool.tile([1, 16], F32)
                nc.gpsimd.memset(warm[:], 0.0)
                nc.gpsimd.memset(warm_in[:], 0.0)
                nc.scalar.activation(warm_y[:], warm_in[:], sigmoid)
                nc.vector.tensor_scalar(
                    warm_v[:], warm_in[:], 0.0, 0.0, add, amax
                )
                NWARM = int(os.environ.get("K_NWARM", "6"))
                for _ in range(NWARM):
                    warm_ps = ps1.tile([HID, 448], F32, tag="p1")
                    nc.tensor.matmul(
                        warm_ps[:], warm[:, :128], warm[:, :448],
                        start=True, stop=True,
                    )

            dataA = sb.tile([128, CUT1], BF16)
            dataB = sb.tile([128, max(CUT2 - CUT1, 1)], BF16)
            dataC = sb.tile([128, max(DCOLS - CUT2, 1)], BF16)
            y1 = sb.tile([1, max(YCUT, 1)], F32)
            y2 = sb.tile([1, NT - YCUT], F32)
            nc.sync.dma_start(dataA[:], data_e[:, :CUT1])
            if CUT2 > CUT1:
                nc.scalar.dma_start(dataB[:], data_e[:, CUT1:CUT2])
            if DCOLS > CUT2:
                nc.sync.dma_start(dataC[:], data_e[:, CUT2:])

            def dcols(c0, c1, r0=0, r1=128):
                if c1 <= CUT1:
                    return dataA[r0:r1, c0:c1]
                if c1 <= CUT2:
                    assert c0 >= CUT1
                    return dataB[r0:r1, c0 - CUT1 : c1 - CUT1]
                assert c0 >= CUT2
                return dataC[r0:r1, c0 - CUT2 : c1 - CUT2]

            def b1_ap(j):
                return dataA[:, 2 * j : 2 * j + 2].bitcast(F32)

            def b2_ap(j):
                c = 2 * S + 2 * j
                return dataA[0:1, c : c + 2].bitcast(F32)

            def yslice(c0, c1):
                if c1 <= YCUT:
                    return y1[:, c0:c1]
                assert c0 >= YCUT
                return y2[:, c0 - YCUT : c1 - YCUT]

            def slot_aps(j):
                p, hi = divmod(j, 2)
                r0 = 64 * hi
                wj = widths[j]
                c0 = XT_OFF + int(pair_offs[p])
                xt = dcols(c0, c0 + wj, r0, r0 + 64)
                w1 = dcols(
                    W1_OFF + p * HID, W1_OFF + (p + 1) * HID, r0, r0 + 64
                )
                return xt, w1

            def mm1(j):
                wj = widths[j]
                xt, w1 = slot_aps(j)
                p1 = ps1.tile([HID, wj], F32, tag="p1")
                nc.tensor.matmul(p1[:], w1, xt, start=True, stop=True)
                return p1

            def relu(j, p1):
                wj = widths[j]
                h = hp.tile([HID, wj], BF16, tag="h")
                nc.vector.tensor_scalar(
                    h[:], p1[:], b1_ap(j), 0.0, add, amax
                )
                return h

            # one PSUM bank per bin; mm2 of each slot writes its column
            # range, one sigmoid per bin reads the whole bank.
            bin_ps = []
            for b in range(NBINS):
                bw = int(sum(widths[j] for j in bins[b]))
                bin_tile = ps2.tile([1, bw], F32, tag=f"bin{b}")
                bin_ps.append(bin_tile)
            bin_left = [len(bs) for bs in bins]

            def mm2(j, h):
                wj = widths[j]
                b = slot_bin[j]
                c0 = int(slot_y_off[j] - bin_off[b])
                nc.tensor.matmul(
                    bin_ps[b][:, c0 : c0 + wj],
                    dcols(W2_OFF + j, W2_OFF + j + 1), h[:],
                    start=True, stop=True,
                )

            def sig_bin(b):
                c0 = int(bin_off[b])
                wb = bin_ps[b].shape[-1]
                bias_ap = b2_ap(0 if b2_uniform else bins[b][0])
                nc.scalar.activation(
                    yslice(c0, c0 + wb), bin_ps[b][:], sigmoid, bias=bias_ap
                )

            def finish_slot(j):
                b = slot_bin[j]
                bin_left[b] -= 1
                if bin_left[b] == 0:
                    sig_bin(b)
                    if YCUT and all(
                        bin_left[bb] == 0 for bb in range(YBINS)
                    ) and b < YBINS:
                        d = nc.sync.dma_start(y_e[:, :YCUT], y1[:])
                        out_dma_names.append(d.ins.name)

            # software-pipelined emission: mm1 of pair p+1 runs on PE while
            # DVE does relu of pair p; mm2 of pair p follows.
            stage = []  # (j, p1)
            for p in range(P + 1):
                if p < P:
                    js = [2 * p] + ([2 * p + 1] if 2 * p + 1 < S else [])
                    nxt = [(j, mm1(j)) for j in js]
                else:
                    nxt = []
                for j, p1 in stage:
                    h = relu(j, p1)
                    mm2(j, h)
                    finish_slot(j)
                stage = nxt

            if YCUT and any(bin_left[bb] > 0 for bb in range(YBINS)):
                d = nc.sync.dma_start(y_e[:, :YCUT], y1[:])
                out_dma_names.append(d.ins.name)
            if YCUT:
                d = nc.sync.dma_start(y_e[:, YCUT:], y2[:])
            else:
                d = nc.sync.dma_start(y_e[:], y2[:])
            out_dma_names.append(d.ins.name)

    _filter_drain_waits(nc, out_dma_names)
    _split_multi_waits(nc)
    return nc


# ---------------------------------------------------------------------------
# Entry point.
# ---------------------------------------------------------------------------
def _run(inputs, trace=False):
    x = np.asarray(inputs["x"], dtype=np.float32)
    num = np.asarray(inputs["num"])
    c = np.asarray(inputs["c"])
    W1 = np.asarray(inputs["W1"], dtype=np.float32)
    b1 = np.asarray(inputs["b1"], dtype=np.float32)
    W2 = np.asarray(inputs["W2"], dtype=np.float32)
    b2 = np.asarray(inputs["b2"], dtype=np.float32)

    B = x.shape[0]
    e = c[num].astype(np.int64)
    b2_uniform = bool(np.all(b2 == b2.flat[0]))
    widths, slots = _plan(e)
    pair_offs, NTP, NT, P, S, bins, bin_off, slot_bin, slot_y_off = _layout(
        widths, MAX_W if b2_uniform else 0
    )
    W1_OFF = 4 * S
    W2_OFF = W1_OFF + P * HID
    XT_OFF = W2_OFF + S
    DCOLS = XT_OFF + NTP

    x_bf = x.astype(NP_BF16)
    W1_bf = W1.astype(NP_BF16)
    W2_bf = W2.astype(NP_BF16)

    in_maps = []
    for core in range(N_CORES):
        data_c = np.zeros((128, DCOLS), dtype=NP_BF16)
        bias_c = np.zeros((128, 2 * S), dtype=np.float32)
        for j in range(S):
            ex, idx = slots[core][j]
            p, hi = divmod(j, 2)
            r0 = 64 * hi
            if len(idx):
                data_c[
                    r0 : r0 + 64,
                    XT_OFF + pair_offs[p] : XT_OFF + pair_offs[p] + len(idx),
                ] = x_bf[idx].T
            data_c[r0 : r0 + 64, W1_OFF + p * HID : W1_OFF + (p + 1) * HID] = (
                W1_bf[ex]
            )
            data_c[:, W2_OFF + j] = W2_bf[ex, :, 0]
            bias_c[:, j] = b1[ex]
            bias_c[0, S + j] = b2[ex, 0]
        data_c[:, : 4 * S] = bias_c.view(NP_BF16)
        in_maps.append({"data": data_c})

    nc = _build(widths, b2_uniform)
    res = run_bass_kernel_spmd(nc, in_maps, list(range(N_CORES)), trace=trace)

    out = np.empty((B, 1), dtype=np.float32)
    for core in range(N_CORES):
        y_c = res.results[core]["y"]
        for j in range(S):
            ex, idx = slots[core][j]
            if len(idx):
                out[idx, 0] = y_c[0, slot_y_off[j] : slot_y_off[j] + len(idx)]
    return out, res


def kernel(**inputs) -> np.ndarray:
    out, _ = _run(inputs, trace=False)
    return out


# revision 35
# speedup vs baseline: 1.8651x; 1.0214x over previous
"""MoE routing kernel for Trainium2 (8 NeuronCores).

Reference computation (B=16384, IN=64, HID=128, OUT=1, E=64, NMAP=1000):
    e = c[num]                                  # [B] expert id per sample
    h = relu(x @ W1[e] + b1[e])                 # [B, HID]
    y = sigmoid(h @ W2[e] + b2[e])              # [B, OUT]

Strategy: sort-by-expert dispatch on the host (the routing is pure
integer bookkeeping), dense per-expert matmuls on device. Each core gets
the same static slot structure (SPMD: one graph for all 8 cores); slot
widths are equalized across cores by snake-dealing the per-expert chunks
in descending size order, padding each slot to the max width over cores.

Device layout (per core): slots are paired onto the 128 SBUF partitions
— pair p puts slot 2p's x^T on partitions 0:64 and slot 2p+1's on
64:128. Full-width DMA, and the two K=64 matmuls of a pair run
concurrently in disjoint PE row groups. Slots are also first-fit packed
into "bins" of <=512 y columns: each bin is one PSUM bank, one
block-diagonal mm2 (lhsT = the bin's w2 columns), and one sigmoid.
All tensor data is bf16 (rel-err budget 2e-2); accumulation stays f32.

Per slot j (width Wj <= 512, pair p, bin b):
    mm1:   psum1[HID=128, Wj] = W1_j[64,128].T @ xT[64, Wj]      (PE)
    relu:  hbin_b[:, cj:cj+Wj] = bf16(max(psum1 + b1_j, 0))      (DVE)
Per bin b (M slots, width Wb <= 512):
    mm2:   psum2[M, Wb] = w2_bin[128,M].T @ hbin_b[128, Wb]      (PE)
    sig:   y[0:M, bin] = sigmoid(psum2 + b2_bin[M,1])            (ACT)
Slot j's outputs live in y[row_of_j_in_bin, its columns] (the
off-diagonal rows are garbage the host ignores).
"""

import os
import sys

if "/opt/trn_rl_repo" not in sys.path:
    sys.path.insert(0, "/opt/trn_rl_repo")

import numpy as np

import concourse.bass as bass
import concourse.mybir as mybir
from concourse import tile
from concourse.bass_utils import run_bass_kernel_spmd

N_CORES = 8
IN = 64
HID = 128
E = 64
MAX_W = 512  # moving-operand / PSUM-bank limit

BF16 = mybir.dt.bfloat16
F32 = mybir.dt.float32
NP_BF16 = mybir.dt.np(BF16)


# ---------------------------------------------------------------------------
# This container's walrus build rejects more than one sync wait per
# instruction ("Too many sync wait commands"). Post-pass over the lowered
# BIR: move the extra waits onto single-wait NOPs inserted just before the
# instruction on the same engine (program order makes this equivalent).
# ---------------------------------------------------------------------------
def _split_multi_waits(nc):
    ctr = 0
    for f in nc.m.functions:
        for blk in f.blocks:
            new_list = []
            for ins in blk.instructions:
                si = ins.sync_info
                if si is not None and si.on_wait and len(si.on_wait) > 1:
                    waits = list(si.on_wait)
                    for w in waits[:-1]:
                        ctr += 1
                        new_list.append(
                            mybir.InstNoOp(
                                name=f"waitsplit-{ctr}",
                                engine=ins.engine,
                                bass_nofuse=True,
                                sync_info=mybir.SyncInfo(
                                    on_wait=[w], on_update=[]
                                ),
                            )
                        )
                    si.on_wait = waits[-1:]
                    ins.sync_info = si
                new_list.append(ins)
            blk.instructions = new_list


def _filter_drain_waits(nc, out_dma_names):
    """The kernel-tail drain only needs to gate on the output DMAs'
    completion semaphores — every other wait Tile put on it is
    transitively implied. Fewer waits = fewer single-wait NOPs."""
    keep_ids = set()
    drain = None
    for f in nc.m.functions:
        for blk in f.blocks:
            for ins in blk.instructions:
                if ins.name in out_dma_names and ins.sync_info is not None:
                    for u in ins.sync_info.on_update:
                        keep_ids.add(u.id)
                if isinstance(ins, mybir.InstDrain):
                    si = ins.sync_info
                    if si is not None and len(si.on_wait) > 1:
                        drain = ins
    if drain is None or not keep_ids:
        return
    si = drain.sync_info
    kept = [w for w in si.on_wait if w.id in keep_ids]
    if kept:
        si.on_wait = kept
        drain.sync_info = si


def _slim_drain_and_barrier(self, tick_clock, wait_clock):
    """Replacement for TileContext._drain_and_barrier: the NEFF here runs
    exactly once per load (run_bass_via_pjrt → single execute), so skip
    the semaphore re-zeroing and the end barriers entirely."""
    drain_inst = self.nc.sync.drain()
    wait_clock.add_sem_waits(
        drain_inst.ins, tile.ScopedClock({None: tick_clock.global_clock})
    )
    popped = self.nc._tile_sem_poison_stack.pop()
    assert popped is self._sem_poison


tile.TileContext._drain_and_barrier = _slim_drain_and_barrier


# ---------------------------------------------------------------------------
# Host-side routing: build the per-core slot structure.
# ---------------------------------------------------------------------------
def _plan(e: np.ndarray):
    """Return (slot_widths, per_core_slots) where per_core_slots[i] is a list
    of (expert_id, sample_indices) aligned with slot_widths (desc order)."""
    order = np.argsort(e, kind="stable")
    counts = np.bincount(e, minlength=max(E, int(e.max()) + 1 if len(e) else E))
    starts = np.concatenate([[0], np.cumsum(counts)])

    chunks = []  # (width, expert, indices)
    for ex in range(len(counts)):
        idx = order[starts[ex] : starts[ex + 1]]
        for pos in range(0, len(idx), MAX_W):
            sub = idx[pos : pos + MAX_W]
            chunks.append((len(sub), ex, sub))
    chunks.sort(key=lambda t: -t[0])

    per_core = [[] for _ in range(N_CORES)]
    for r in range(0, len(chunks), N_CORES):
        row = chunks[r : r + N_CORES]
        cores = range(N_CORES) if (r // N_CORES) % 2 == 0 else range(N_CORES - 1, -1, -1)
        for ch, core in zip(row, cores):
            per_core[core].append(ch)

    n_slots = max(len(s) for s in per_core)
    empty = np.zeros((0,), dtype=np.int64)
    for s in per_core:
        while len(s) < n_slots:
            s.append((0, 0, empty))
        s.sort(key=lambda t: -t[0])

    widths = [max(per_core[i][j][0] for i in range(N_CORES)) for j in range(n_slots)]
    widths = [max(w, 1) for w in widths]
    slots = [[(s[j][1], s[j][2]) for j in range(n_slots)] for s in per_core]
    return widths, slots


class _Layout:
    """Column layout shared by the graph builder and the host packer.

    data tensor (bf16 cols):
      [0, 2S)          b1 columns, f32 bitcast (col j = b1 of slot j)
      [2S, 2S+2NB)     b2 columns, f32 bitcast (col b, partition i = b2 of
                       bins[b][i])
      [HDR, ...)       per pair p: W1_p (HID cols, slot 2p on partitions
                       0:64, slot 2p+1 on 64:128) then xT_p (pw_p cols,
                       same stacking); pairs 0, 1, 2.. in order
      [W2_OFF, +S)     w2 columns in bin order (col slot_pos[j])
    Input DMA split: A = header + w2 + pair0 (sync), B = pair1 (scalar),
    C = pairs 2.. (sync).
    """

    def __init__(self, widths):
        S = len(widths)
        P = (S + 1) // 2
        self.widths = widths
        self.S, self.P = S, P
        self.pw = [widths[2 * p] + (widths[2 * p] & 1) for p in range(P)]
        self.NT = int(np.sum(widths))

        bins, bin_w = [], []
        self.slot_bin = [0] * S
        for j in range(S):
            for b in range(len(bins)):
                if bin_w[b] + widths[j] <= MAX_W:
                    bins[b].append(j)
                    bin_w[b] += widths[j]
                    self.slot_bin[j] = b
                    break
            else:
                self.slot_bin[j] = len(bins)
                bins.append([j])
                bin_w.append(widths[j])
        self.bins, self.bin_w = bins, bin_w
        self.NB = len(bins)
        self.Mmax = max(len(bs) for bs in bins)

        self.slot_y_off = [0] * S  # column in y / position of slot's range
        self.slot_row = [0] * S  # row in y
        self.slot_pos = [0] * S  # w2 column
        self.bin_off = []
        off = pos = 0
        for b, bs in enumerate(bins):
            self.bin_off.append(off)
            for i, j in enumerate(bs):
                self.slot_y_off[j] = off
                self.slot_row[j] = i
                self.slot_pos[j] = pos
                off += widths[j]
                pos += 1
        assert off == self.NT

        self.HDR = 2 * S + 2 * self.NB
        self.pair_base = []
        c = self.HDR
        for p in range(P):
            self.pair_base.append(c)
            c += HID + self.pw[p]
        self.W2_OFF = c
        self.DCOLS = c + S + (S & 1)
        self.CUT1 = self.pair_base[1] if P > 1 else self.W2_OFF
        self.CUT2 = self.pair_base[2] if P > 2 else self.W2_OFF
        # y DMA split: y1 covers the first YBINS bins
        self.YBINS = max(1, self.NB // 2) if self.NB > 1 else 0
        self.YCUT = self.bin_off[self.YBINS] if self.YBINS else 0

    def w1_cols(self, j):
        p = j // 2
        return self.pair_base[p], self.pair_base[p] + HID

    def xt_cols(self, j):
        p = j // 2
        c0 = self.pair_base[p] + HID
        return c0, c0 + self.widths[j]


# ---------------------------------------------------------------------------
# Device graph builder (shared by all cores).
# ---------------------------------------------------------------------------
def _build(L: _Layout):
    S, P, NB = L.S, L.P, L.NB
    widths = L.widths

    nc = bass.Bass("TRN2", target_bir_lowering=False, debug=False)
    data_e = nc.declare_dram_parameter("data", [128, L.DCOLS], BF16, isOutput=False)
    y_e = nc.declare_dram_parameter("y", [L.Mmax, L.NT], F32, isOutput=True)

    sigmoid = mybir.ActivationFunctionType.Sigmoid
    add = mybir.AluOpType.add
    amax = mybir.AluOpType.max

    out_dma_names = []
    with tile.TileContext(nc) as tc:
        with (
            tc.tile_pool(name="sb", bufs=1) as sb,
            tc.tile_pool(
                name="ps1", bufs=max(1, min(3, 8 - NB)), space="PSUM"
            ) as ps1,
            tc.tile_pool(name="ps2", bufs=1, space="PSUM") as ps2,
            tc.tile_pool(name="dummy", bufs=1) as dummy_pool,
        ):
            # Engine preloads during the input DMA window (all on garbage
            # SBUF, no data deps): ACT sigmoid table load, DVE first-op
            # cost, PE pipeline priming. The warmup matmuls rotate through
            # the same psum bufs the real mm1s use (PE executes in order).
            WARMUP = os.environ.get("K_WARMUP", "1") == "1"
            if WARMUP:
                warm = dummy_pool.tile([128, 512], BF16)
                warm_in = dummy_pool.tile([1, 16], F32)
                warm_y = dummy_pool.tile([1, 16], F32)
                warm_v = dummy_pool.tile([1, 16], F32)
                nc.gpsimd.memset(warm[:], 0.0)
                nc.gpsimd.memset(warm_in[:], 0.0)
                nc.scalar.activation(warm_y[:], warm_in[:], sigmoid)
                nc.vector.tensor_scalar(
                    warm_v[:], warm_in[:], 0.0, 0.0, add, amax
                )
                for _ in range(int(os.environ.get("K_NWARM", "4"))):
                    warm_ps = ps1.tile([HID, 448], F32, tag="p1")
                    nc.tensor.matmul(
                        warm_ps[:], warm[:, :128], warm[:, :448],
                        start=True, stop=True,
                    )

            dataA = sb.tile([128, L.CUT1], BF16)
            dataB = sb.tile([128, max(L.CUT2 - L.CUT1, 1)], BF16)
            dataC = sb.tile([128, max(L.DCOLS - L.CUT2, 1)], BF16)
            y1 = sb.tile([L.Mmax, max(L.YCUT, 1)], F32)
            y2 = sb.tile([L.Mmax, L.NT - L.YCUT], F32)
            hbin = []
            for b in range(NB):
                hb = sb.tile([HID, L.bin_w[b]], BF16, tag=f"h{b}")
                hbin.append(hb)

            nc.sync.dma_start(dataA[:], data_e[:, : L.CUT1])
            if L.CUT2 > L.CUT1:
                nc.scalar.dma_start(dataB[:], data_e[:, L.CUT1 : L.CUT2])
            if L.DCOLS > L.CUT2:
                nc.sync.dma_start(dataC[:], data_e[:, L.CUT2 :])

            def dcols(c0, c1, r0=0, r1=128):
                if c1 <= L.CUT1:
                    return dataA[r0:r1, c0:c1]
                if c1 <= L.CUT2:
                    assert c0 >= L.CUT1
                    return dataB[r0:r1, c0 - L.CUT1 : c1 - L.CUT1]
                assert c0 >= L.CUT2
                return dataC[r0:r1, c0 - L.CUT2 : c1 - L.CUT2]

            def b1_ap(j):
                return dataA[:, 2 * j : 2 * j + 2].bitcast(F32)

            def b2_ap(b, m):
                c = 2 * S + 2 * b
                return dataA[0:m, c : c + 2].bitcast(F32)

            def yslice(r, c0, c1):
                if c1 <= L.YCUT:
                    return y1[0:r, c0:c1]
                assert c0 >= L.YCUT
                return y2[0:r, c0 - L.YCUT : c1 - L.YCUT]

            def mm1(j):
                p, hi = divmod(j, 2)
                r0 = 64 * hi
                c0, c1 = L.xt_cols(j)
                w0, w1c = L.w1_cols(j)
                p1 = ps1.tile([HID, widths[j]], F32, tag="p1")
                nc.tensor.matmul(
                    p1[:],
                    dcols(w0, w1c, r0, r0 + 64),
                    dcols(c0, c1, r0, r0 + 64),
                    start=True,
                    stop=True,
                )
                return p1

            def relu(j, p1):
                b = L.slot_bin[j]
                c0 = L.slot_y_off[j] - L.bin_off[b]
                nc.vector.tensor_scalar(
                    hbin[b][:, c0 : c0 + widths[j]],
                    p1[:],
                    b1_ap(j),
                    0.0,
                    add,
                    amax,
                )

            bin_ps = []
            for b in range(NB):
                m = len(L.bins[b])
                bp = ps2.tile([m, L.bin_w[b]], F32, tag=f"bin{b}")
                bin_ps.append(bp)
            bin_left = [len(bs) for bs in L.bins]

            def finish_bin(b):
                m = len(L.bins[b])
                p0 = L.slot_pos[L.bins[b][0]]
                nc.tensor.matmul(
                    bin_ps[b][:],
                    dcols(L.W2_OFF + p0, L.W2_OFF + p0 + m),
                    hbin[b][:],
                    start=True,
                    stop=True,
                )
                c0 = L.bin_off[b]
                nc.scalar.activation(
                    yslice(m, c0, c0 + L.bin_w[b]),
                    bin_ps[b][:],
                    sigmoid,
                    bias=b2_ap(b, m),
                )

            def finish_slot(j):
                b = L.slot_bin[j]
                bin_left[b] -= 1
                if bin_left[b] == 0:
                    finish_bin(b)
                    if L.YCUT and all(
                        bin_left[bb] == 0 for bb in range(L.YBINS)
                    ) and b < L.YBINS:
                        d = nc.sync.dma_start(y_e[:, : L.YCUT], y1[:])
                        out_dma_names.append(d.ins.name)

            # software-pipelined emission: mm1 of pair p+1 runs on PE while
            # DVE does relu of pair p; bin mm2s/sigmoids fire as bins fill.
            stage = []  # (j, p1)
            for p in range(P + 1):
                if p < P:
                    js = [2 * p] + ([2 * p + 1] if 2 * p + 1 < S else [])
                    nxt = [(j, mm1(j)) for j in js]
                else:
                    nxt = []
                for j, p1 in stage:
                    relu(j, p1)
                    finish_slot(j)
                stage = nxt

            if L.YCUT and any(bin_left[bb] > 0 for bb in range(L.YBINS)):
                d = nc.sync.dma_start(y_e[:, : L.YCUT], y1[:])
                out_dma_names.append(d.ins.name)
            if L.YCUT:
                d = nc.sync.dma_start(y_e[:, L.YCUT :], y2[:])
            else:
                d = nc.sync.dma_start(y_e[:], y2[:])
            out_dma_names.append(d.ins.name)

    _filter_drain_waits(nc, out_dma_names)
    _split_multi_waits(nc)
    return nc


# ---------------------------------------------------------------------------
# Entry point.
# ---------------------------------------------------------------------------
def _run(inputs, trace=False):
    x = np.asarray(inputs["x"], dtype=np.float32)
    num = np.asarray(inputs["num"])
    c = np.asarray(inputs["c"])
    W1 = np.asarray(inputs["W1"], dtype=np.float32)
    b1 = np.asarray(inputs["b1"], dtype=np.float32)
    W2 = np.asarray(inputs["W2"], dtype=np.float32)
    b2 = np.asarray(inputs["b2"], dtype=np.float32)

    B = x.shape[0]
    e = c[num].astype(np.int64)
    widths, slots = _plan(e)
    L = _Layout(widths)
    S = L.S

    x_bf = x.astype(NP_BF16)
    W1_bf = W1.astype(NP_BF16)
    W2_bf = W2.astype(NP_BF16)

    in_maps = []
    for core in range(N_CORES):
        data_c = np.zeros((128, L.DCOLS), dtype=NP_BF16)
        b1_c = np.zeros((128, S), dtype=np.float32)
        b2_c = np.zeros((128, L.NB), dtype=np.float32)
        for j in range(S):
            ex, idx = slots[core][j]
            p, hi = divmod(j, 2)
            r0 = 64 * hi
            w0, w1c = L.w1_cols(j)
            c0, _ = L.xt_cols(j)
            if len(idx):
                data_c[r0 : r0 + 64, c0 : c0 + len(idx)] = x_bf[idx].T
            data_c[r0 : r0 + 64, w0:w1c] = W1_bf[ex]
            data_c[:, L.W2_OFF + L.slot_pos[j]] = W2_bf[ex, :, 0]
            b1_c[:, j] = b1[ex]
            b2_c[L.slot_row[j], L.slot_bin[j]] = b2[ex, 0]
        data_c[:, : 2 * S] = b1_c.view(NP_BF16)
        data_c[:, 2 * S : 2 * S + 2 * L.NB] = b2_c.view(NP_BF16)
        in_maps.append({"data": data_c})

    nc = _build(L)
    res = run_bass_kernel_spmd(nc, in_maps, list(range(N_CORES)), trace=trace)

    out = np.empty((B, 1), dtype=np.float32)
    for core in range(N_CORES):
        y_c = res.results[core]["y"]
        for j in range(S):
            ex, idx = slots[core][j]
            if len(idx):
                out[idx, 0] = y_c[
                    L.slot_row[j], L.slot_y_off[j] : L.slot_y_off[j] + len(idx)
                ]
    return out, res


def kernel(**inputs) -> np.ndarray:
    out, _ = _run(inputs, trace=False)
    return out


# revision 39
# speedup vs baseline: 1.9081x; 1.0231x over previous
"""MoE routing kernel for Trainium2 (8 NeuronCores).

Reference computation (B=16384, IN=64, HID=128, OUT=1, E=64, NMAP=1000):
    e = c[num]                                  # [B] expert id per sample
    h = relu(x @ W1[e] + b1[e])                 # [B, HID]
    y = sigmoid(h @ W2[e] + b2[e])              # [B, OUT]

Strategy: sort-by-expert dispatch on the host (the routing is pure
integer bookkeeping), dense per-expert matmuls on device. Each core gets
the same static slot structure (SPMD: one graph for all 8 cores); slot
widths are equalized across cores by snake-dealing the per-expert chunks
in descending size order, padding each slot to the max width over cores.

Device layout (per core): slots are paired onto the 128 SBUF partitions
— pair p puts slot 2p's x^T on partitions 0:64 and slot 2p+1's on
64:128. Full-width DMA, and the two K=64 matmuls of a pair run
concurrently in disjoint PE row groups. Slots are also first-fit packed
into "bins" of <=512 y columns: each bin is one PSUM bank, one
block-diagonal mm2 (lhsT = the bin's w2 columns), and one sigmoid.
All tensor data is bf16 (rel-err budget 2e-2); accumulation stays f32.

Per slot j (width Wj <= 512, pair p, bin b):
    mm1:   psum1[HID=128, Wj] = W1_j[64,128].T @ xT[64, Wj]      (PE)
    relu:  hbin_b[:, cj:cj+Wj] = bf16(max(psum1 + b1_j, 0))      (DVE)
Per bin b (M slots, width Wb <= 512):
    mm2:   psum2[M, Wb] = w2_bin[128,M].T @ hbin_b[128, Wb]      (PE)
    sig:   y[0:M, bin] = sigmoid(psum2 + b2_bin[M,1])            (ACT)
Slot j's outputs live in y[row_of_j_in_bin, its columns] (the
off-diagonal rows are garbage the host ignores).
"""

import os
import sys

if "/opt/trn_rl_repo" not in sys.path:
    sys.path.insert(0, "/opt/trn_rl_repo")

import numpy as np

import concourse.bass as bass
import concourse.mybir as mybir
from concourse import tile
from concourse.bass_utils import run_bass_kernel_spmd

N_CORES = 8
IN = 64
HID = 128
E = 64
MAX_W = 512  # moving-operand / PSUM-bank limit

BF16 = mybir.dt.bfloat16
F32 = mybir.dt.float32
NP_BF16 = mybir.dt.np(BF16)


# ---------------------------------------------------------------------------
# This container's walrus build rejects more than one sync wait per
# instruction ("Too many sync wait commands"). Post-pass over the lowered
# BIR: move the extra waits onto single-wait NOPs inserted just before the
# instruction on the same engine (program order makes this equivalent).
# ---------------------------------------------------------------------------
def _split_multi_waits(nc):
    ctr = 0
    for f in nc.m.functions:
        for blk in f.blocks:
            new_list = []
            for ins in blk.instructions:
                si = ins.sync_info
                if si is not None and si.on_wait and len(si.on_wait) > 1:
                    waits = list(si.on_wait)
                    for w in waits[:-1]:
                        ctr += 1
                        new_list.append(
                            mybir.InstNoOp(
                                name=f"waitsplit-{ctr}",
                                engine=ins.engine,
                                bass_nofuse=True,
                                sync_info=mybir.SyncInfo(
                                    on_wait=[w], on_update=[]
                                ),
                            )
                        )
                    si.on_wait = waits[-1:]
                    ins.sync_info = si
                new_list.append(ins)
            blk.instructions = new_list


def _filter_drain_waits(nc, out_dma_names):
    """The kernel-tail drain only needs to gate on the output DMAs'
    completion semaphores — every other wait Tile put on it is
    transitively implied. Fewer waits = fewer single-wait NOPs."""
    keep_ids = set()
    drain = None
    for f in nc.m.functions:
        for blk in f.blocks:
            for ins in blk.instructions:
                if ins.name in out_dma_names and ins.sync_info is not None:
                    for u in ins.sync_info.on_update:
                        keep_ids.add(u.id)
                if isinstance(ins, mybir.InstDrain):
                    si = ins.sync_info
                    if si is not None and len(si.on_wait) > 1:
                        drain = ins
    if drain is None or not keep_ids:
        return
    si = drain.sync_info
    kept = [w for w in si.on_wait if w.id in keep_ids]
    if kept:
        si.on_wait = kept
        drain.sync_info = si


def _slim_drain_and_barrier(self, tick_clock, wait_clock):
    """Replacement for TileContext._drain_and_barrier: the NEFF here runs
    exactly once per load (run_bass_via_pjrt → single execute), so skip
    the semaphore re-zeroing and the end barriers entirely."""
    drain_inst = self.nc.sync.drain()
    wait_clock.add_sem_waits(
        drain_inst.ins, tile.ScopedClock({None: tick_clock.global_clock})
    )
    popped = self.nc._tile_sem_poison_stack.pop()
    assert popped is self._sem_poison


tile.TileContext._drain_and_barrier = _slim_drain_and_barrier


# ---------------------------------------------------------------------------
# Host-side routing: build the per-core slot structure.
# ---------------------------------------------------------------------------
def _plan(e: np.ndarray):
    """Return (slot_widths, per_core_slots) where per_core_slots[i] is a list
    of (expert_id, sample_indices) aligned with slot_widths (desc order)."""
    order = np.argsort(e, kind="stable")
    counts = np.bincount(e, minlength=max(E, int(e.max()) + 1 if len(e) else E))
    starts = np.concatenate([[0], np.cumsum(counts)])

    chunks = []  # (width, expert, indices)
    for ex in range(len(counts)):
        idx = order[starts[ex] : starts[ex + 1]]
        for pos in range(0, len(idx), MAX_W):
            sub = idx[pos : pos + MAX_W]
            chunks.append((len(sub), ex, sub))
    chunks.sort(key=lambda t: -t[0])

    per_core = [[] for _ in range(N_CORES)]
    for r in range(0, len(chunks), N_CORES):
        row = chunks[r : r + N_CORES]
        cores = range(N_CORES) if (r // N_CORES) % 2 == 0 else range(N_CORES - 1, -1, -1)
        for ch, core in zip(row, cores):
            per_core[core].append(ch)

    n_slots = max(len(s) for s in per_core)
    empty = np.zeros((0,), dtype=np.int64)
    for s in per_core:
        while len(s) < n_slots:
            s.append((0, 0, empty))
        s.sort(key=lambda t: -t[0])

    widths = [max(per_core[i][j][0] for i in range(N_CORES)) for j in range(n_slots)]
    widths = [max(w, 1) for w in widths]
    slots = [[(s[j][1], s[j][2]) for j in range(n_slots)] for s in per_core]
    return widths, slots


class _Layout:
    """Column layout shared by the graph builder and the host packer.

    data tensor (bf16 cols):
      [0, 2S)          b1 columns, f32 bitcast (col j = b1 of slot j)
      [2S, 2S+2NB)     b2 columns, f32 bitcast (col b, partition i = b2 of
                       bins[b][i])
      [HDR, ...)       per pair p: W1_p (HID cols, slot 2p on partitions
                       0:64, slot 2p+1 on 64:128) then xT_p (pw_p cols,
                       same stacking); pairs 0, 1, 2.. in order
      [W2_OFF, +S)     w2 columns in bin order (col slot_pos[j])
    Input DMA split: A = header + w2 + pair0 (sync), B = pair1 (scalar),
    C = pairs 2.. (sync).
    """

    def __init__(self, widths):
        S = len(widths)
        P = (S + 1) // 2
        self.widths = widths
        self.S, self.P = S, P
        self.NT = int(np.sum(widths))

        bins, bin_w = [], []
        self.slot_bin = [0] * S
        for j in range(S):
            for b in range(len(bins)):
                if bin_w[b] + widths[j] <= MAX_W:
                    bins[b].append(j)
                    bin_w[b] += widths[j]
                    self.slot_bin[j] = b
                    break
            else:
                self.slot_bin[j] = len(bins)
                bins.append([j])
                bin_w.append(widths[j])
        self.bins, self.bin_w = bins, bin_w
        self.NB = len(bins)
        self.Mmax = max(len(bs) for bs in bins)

        # slot processing order = bin order, so bins complete (and their
        # mm2+sigmoid fire) sequentially instead of piling up at the end
        self.proc = [j for bs in bins for j in bs]

        self.slot_y_off = [0] * S  # column in y / position of slot's range
        self.slot_row = [0] * S  # row in y
        self.slot_pos = [0] * S  # w2 column
        self.bin_off = []
        off = pos = 0
        for b, bs in enumerate(bins):
            self.bin_off.append(off)
            for i, j in enumerate(bs):
                self.slot_y_off[j] = off
                self.slot_row[j] = i
                self.slot_pos[j] = pos
                off += widths[j]
                pos += 1
        assert off == self.NT

        # pairs follow the processing order: pair k stacks proc[2k] on
        # partitions 0:64 and proc[2k+1] on 64:128
        self.pair_of = {}
        self.hi_of = {}
        self.pairs = []
        for k in range(P):
            js = self.proc[2 * k : 2 * k + 2]
            self.pairs.append(js)
            for hi, j in enumerate(js):
                self.pair_of[j] = k
                self.hi_of[j] = hi
        self.pw = [
            max(widths[j] for j in js) + (max(widths[j] for j in js) & 1)
            for js in self.pairs
        ]

        self.HDR = 2 * S + 2 * self.NB
        self.pair_base = []
        c = self.HDR
        for k in range(P):
            self.pair_base.append(c)
            c += HID + self.pw[k]
        self.W2_OFF = c
        self.DCOLS = c + S + (S & 1)
        self.CUT1 = self.pair_base[1] if P > 1 else self.W2_OFF
        self.CUT2 = self.pair_base[2] if P > 2 else self.W2_OFF
        # y DMA split: y1 covers the first YBINS bins
        self.YBINS = max(1, self.NB // 2) if self.NB > 1 else 0
        self.YCUT = self.bin_off[self.YBINS] if self.YBINS else 0

    def w1_cols(self, j):
        p = self.pair_of[j]
        return self.pair_base[p], self.pair_base[p] + HID

    def xt_cols(self, j):
        p = self.pair_of[j]
        c0 = self.pair_base[p] + HID
        return c0, c0 + self.widths[j]


# ---------------------------------------------------------------------------
# Device graph builder (shared by all cores).
# ---------------------------------------------------------------------------
def _build(L: _Layout):
    S, P, NB = L.S, L.P, L.NB
    widths = L.widths

    nc = bass.Bass("TRN2", target_bir_lowering=False, debug=False)
    data_e = nc.declare_dram_parameter("data", [128, L.DCOLS], BF16, isOutput=False)
    y_e = nc.declare_dram_parameter("y", [L.Mmax, L.NT], F32, isOutput=True)

    sigmoid = mybir.ActivationFunctionType.Sigmoid
    add = mybir.AluOpType.add
    amax = mybir.AluOpType.max

    out_dma_names = []
    with tile.TileContext(nc) as tc:
        with (
            tc.tile_pool(name="sb", bufs=1) as sb,
            tc.tile_pool(
                name="ps1", bufs=max(1, min(3, 8 - NB)), space="PSUM"
            ) as ps1,
            tc.tile_pool(name="ps2", bufs=1, space="PSUM") as ps2,
            tc.tile_pool(name="dummy", bufs=1) as dummy_pool,
        ):
            # Engine preloads during the input DMA window (all on garbage
            # SBUF, no data deps): ACT sigmoid table load, DVE first-op
            # cost, PE pipeline priming. The warmup matmuls rotate through
            # the same psum bufs the real mm1s use (PE executes in order).
            WARMUP = os.environ.get("K_WARMUP", "1") == "1"
            if WARMUP:
                warm = dummy_pool.tile([128, 512], BF16)
                warm_in = dummy_pool.tile([1, 16], F32)
                warm_y = dummy_pool.tile([1, 16], F32)
                warm_v = dummy_pool.tile([1, 16], F32)
                nc.gpsimd.memset(warm[:], 0.0)
                nc.gpsimd.memset(warm_in[:], 0.0)
                nc.scalar.activation(warm_y[:], warm_in[:], sigmoid)
                nc.vector.tensor_scalar(
                    warm_v[:], warm_in[:], 0.0, 0.0, add, amax
                )
                for _ in range(int(os.environ.get("K_NWARM", "4"))):
                    warm_ps = ps1.tile([HID, 448], F32, tag="p1")
                    nc.tensor.matmul(
                        warm_ps[:], warm[:, :128], warm[:, :448],
                        start=True, stop=True,
                    )

            dataA = sb.tile([128, L.CUT1], BF16)
            dataB = sb.tile([128, max(L.CUT2 - L.CUT1, 1)], BF16)
            dataC = sb.tile([128, max(L.DCOLS - L.CUT2, 1)], BF16)
            y1 = sb.tile([L.Mmax, max(L.YCUT, 1)], F32)
            y2 = sb.tile([L.Mmax, L.NT - L.YCUT], F32)
            hbin = []
            for b in range(NB):
                hb = sb.tile([HID, L.bin_w[b]], BF16, tag=f"h{b}")
                hbin.append(hb)

            nc.sync.dma_start(dataA[:], data_e[:, : L.CUT1])
            if L.CUT2 > L.CUT1:
                nc.scalar.dma_start(dataB[:], data_e[:, L.CUT1 : L.CUT2])
            if L.DCOLS > L.CUT2:
                nc.sync.dma_start(dataC[:], data_e[:, L.CUT2 :])

            def dcols(c0, c1, r0=0, r1=128):
                if c1 <= L.CUT1:
                    return dataA[r0:r1, c0:c1]
                if c1 <= L.CUT2:
                    assert c0 >= L.CUT1
                    return dataB[r0:r1, c0 - L.CUT1 : c1 - L.CUT1]
                assert c0 >= L.CUT2
                return dataC[r0:r1, c0 - L.CUT2 : c1 - L.CUT2]

            def b1_ap(j):
                return dataA[:, 2 * j : 2 * j + 2].bitcast(F32)

            def b2_ap(b, m):
                c = 2 * S + 2 * b
                return dataA[0:m, c : c + 2].bitcast(F32)

            def yslice(r, c0, c1):
                if c1 <= L.YCUT:
                    return y1[0:r, c0:c1]
                assert c0 >= L.YCUT
                return y2[0:r, c0 - L.YCUT : c1 - L.YCUT]

            def mm1(j):
                r0 = 64 * L.hi_of[j]
                c0, c1 = L.xt_cols(j)
                w0, w1c = L.w1_cols(j)
                p1 = ps1.tile([HID, widths[j]], F32, tag="p1")
                nc.tensor.matmul(
                    p1[:],
                    dcols(w0, w1c, r0, r0 + 64),
                    dcols(c0, c1, r0, r0 + 64),
                    start=True,
                    stop=True,
                )
                return p1

            def relu(j, p1):
                b = L.slot_bin[j]
                c0 = L.slot_y_off[j] - L.bin_off[b]
                nc.vector.tensor_scalar(
                    hbin[b][:, c0 : c0 + widths[j]],
                    p1[:],
                    b1_ap(j),
                    0.0,
                    add,
                    amax,
                )

            bin_ps = []
            for b in range(NB):
                m = len(L.bins[b])
                bp = ps2.tile([m, L.bin_w[b]], F32, tag=f"bin{b}")
                bin_ps.append(bp)
            bin_left = [len(bs) for bs in L.bins]

            def finish_bin(b):
                m = len(L.bins[b])
                p0 = L.slot_pos[L.bins[b][0]]
                nc.tensor.matmul(
                    bin_ps[b][:],
                    dcols(L.W2_OFF + p0, L.W2_OFF + p0 + m),
                    hbin[b][:],
                    start=True,
                    stop=True,
                )
                c0 = L.bin_off[b]
                nc.scalar.activation(
                    yslice(m, c0, c0 + L.bin_w[b]),
                    bin_ps[b][:],
                    sigmoid,
                    bias=b2_ap(b, m),
                )

            def finish_slot(j):
                b = L.slot_bin[j]
                bin_left[b] -= 1
                if bin_left[b] == 0:
                    finish_bin(b)
                    if L.YCUT and all(
                        bin_left[bb] == 0 for bb in range(L.YBINS)
                    ) and b < L.YBINS:
                        d = nc.sync.dma_start(y_e[:, : L.YCUT], y1[:])
                        out_dma_names.append(d.ins.name)

            # software-pipelined emission: mm1 of pair p+1 runs on PE while
            # DVE does relu of pair p; bin mm2s/sigmoids fire as bins fill.
            stage = []  # (j, p1)
            for p in range(P + 1):
                if p < P:
                    nxt = [(j, mm1(j)) for j in L.pairs[p]]
                else:
                    nxt = []
                for j, p1 in stage:
                    relu(j, p1)
                    finish_slot(j)
                stage = nxt

            if L.YCUT and any(bin_left[bb] > 0 for bb in range(L.YBINS)):
                d = nc.sync.dma_start(y_e[:, : L.YCUT], y1[:])
                out_dma_names.append(d.ins.name)
            if L.YCUT:
                d = nc.sync.dma_start(y_e[:, L.YCUT :], y2[:])
            else:
                d = nc.sync.dma_start(y_e[:], y2[:])
            out_dma_names.append(d.ins.name)

    _filter_drain_waits(nc, out_dma_names)
    _split_multi_waits(nc)
    return nc


# ---------------------------------------------------------------------------
# Entry point.
# ---------------------------------------------------------------------------
def _run(inputs, trace=False):
    x = np.asarray(inputs["x"], dtype=np.float32)
    num = np.asarray(inputs["num"])
    c = np.asarray(inputs["c"])
    W1 = np.asarray(inputs["W1"], dtype=np.float32)
    b1 = np.asarray(inputs["b1"], dtype=np.float32)
    W2 = np.asarray(inputs["W2"], dtype=np.float32)
    b2 = np.asarray(inputs["b2"], dtype=np.float32)

    B = x.shape[0]
    e = c[num].astype(np.int64)
    widths, slots = _plan(e)
    L = _Layout(widths)
    S = L.S

    x_bf = x.astype(NP_BF16)
    W1_bf = W1.astype(NP_BF16)
    W2_bf = W2.astype(NP_BF16)

    in_maps = []
    for core in range(N_CORES):
        data_c = np.zeros((128, L.DCOLS), dtype=NP_BF16)
        b1_c = np.zeros((128, S), dtype=np.float32)
        b2_c = np.zeros((128, L.NB), dtype=np.float32)
        for j in range(S):
            ex, idx = slots[core][j]
            r0 = 64 * L.hi_of[j]
            w0, w1c = L.w1_cols(j)
            c0, _ = L.xt_cols(j)
            if len(idx):
                data_c[r0 : r0 + 64, c0 : c0 + len(idx)] = x_bf[idx].T
            data_c[r0 : r0 + 64, w0:w1c] = W1_bf[ex]
            data_c[:, L.W2_OFF + L.slot_pos[j]] = W2_bf[ex, :, 0]
            b1_c[:, j] = b1[ex]
            b2_c[L.slot_row[j], L.slot_bin[j]] = b2[ex, 0]
        data_c[:, : 2 * S] = b1_c.view(NP_BF16)
        data_c[:, 2 * S : 2 * S + 2 * L.NB] = b2_c.view(NP_BF16)
        in_maps.append({"data": data_c})

    nc = _build(L)
    res = run_bass_kernel_spmd(nc, in_maps, list(range(N_CORES)), trace=trace)

    out = np.empty((B, 1), dtype=np.float32)
    for core in range(N_CORES):
        y_c = res.results[core]["y"]
        for j in range(S):
            ex, idx = slots[core][j]
            if len(idx):
                out[idx, 0] = y_c[
                    L.slot_row[j], L.slot_y_off[j] : L.slot_y_off[j] + len(idx)
                ]
    return out, res


def kernel(**inputs) -> np.ndarray:
    out, _ = _run(inputs, trace=False)
    return out
